# revision 76
# baseline (speedup 1.0000x reference)
"""BinarizedConvNet forward on 8 Trainium2 cores, v3.

Measured-hardware facts this version targets:
  - PE matmul cost ~ (free_size x pe_cycle) + ~12-15ns per rhs AP row;
    LDWEIGHTS overlaps on its own pipe; clock ramps with continuous use.
  - DMA: ~8 GB/s per channel x 16 channels ~ 127 GB/s aggregate,
    regardless of descriptor size or HW/SW DGE. Input bytes are the
    conv1 floor, so only the raw x rows ship (6.1MB + 2MB zero spacer);
    the x-shifted copy for odd-dx taps is built on-chip by DVE at
    stride 2 (only even columns of the shifted rows are ever read).
  - First collective pays ~90us mesh warmup: a dummy AllReduce issued
    at t~0 absorbs it (and most rank-start skew) under conv1.
  - TRN2 instructions carry at most ONE semaphore wait; a post-pass
    strips same-engine and dominated waits and relocates a second
    matmul wait onto its preceding LDWEIGHTS.

Structure:
  conv1: K=56 rows per group = 24 raw + 8 zero spacer + 24 x-shifted
    (engine ops need 32-aligned partition starts; the zero weights over
    the spacer keep PE's IEEE 0*NaN poison away). 4 groups of 8 imgs,
    4 taps of K=56 per (img, rowclass), outputs class-packed at psum
    partitions 32*c3.
  conv2: per dx one K=96 MM (dy 0,1,2) + one K=32 MM (dy=3): 8 MMs per
    4-img chunk; relu1 is bias-only on DVE (bn1 scale folded into w2).
  conv3: 36 dense K=64 MMs, 8 imgs per chunk; relu2 on DVE.
  fc1: PE transposes + oc-group matmuls + indicator-sum matmul.
  fc tail: ONE AllGather of the fc1 pre-activations (z4), then
    bnf1 -> fc2 -> bnf2 -> fc3 computed for the FULL batch on every
    core (output [256, 9], identical across cores; buffers alias dead
    xq columns).

bn1/bn2/bn3 use exact global batch stats via AllReduce of (sum, sumsq)
with gamma/beta pre-divided by NCORES; bnf1/bnf2 are local after the
AllGather (gamma/beta undivided).
"""

import numpy as np
import ml_dtypes

import concourse.bass as bass
import concourse.mybir as mybir
import concourse.tile as tile
import concourse.tile_sem_assignment as _tsa
from concourse.tile_rust import add_dep_helper
from concourse.bass_utils import run_bass_kernel_spmd

_tsa.NUM_SWDGE_GLOBAL_SEMS = 1

dt = mybir.dt
BF, F32 = dt.bfloat16, dt.float32
AF = mybir.ActivationFunctionType
ALU = mybir.AluOpType
bf16 = ml_dtypes.bfloat16

NCORES = 8
Bc = 32
EPS = 1e-5
B = 256

N1 = B * 31 * 31
N2 = B * 10 * 10
N3 = B * 8 * 8
N4 = B
N5 = B

C1_NY = [11, 10, 10]   # conv1 class row counts (y' = c3 + 3k)


def build_program(ncores=NCORES):
    nc = bass.Bass()

    # xin rows 32g..32g+32 = group g (8 imgs): 24 xr k-rows + 8 zero
    # spacer rows, free = (img 8, y 31, x 128) contiguous per partition.
    # The x-shifted copy (odd-dx taps) is built on-chip: DMA BW
    # (~127 GB/s aggregate) is the conv1 floor, so shrinking DRAM bytes
    # wins even at the cost of an on-chip 24-lane copy per group. The
    # zero spacer keeps every SBUF K row initialized (no NaN garbage
    # under the zero weight rows) while keeping engine-op partition
    # starts 32-aligned.
    xin = nc.declare_dram_parameter("xin", [128, 8 * 31 * 128], BF,
                                    isOutput=False)
    bfblob = nc.declare_dram_parameter("bfblob", [128, 1600], BF, isOutput=False)
    fblob = nc.declare_dram_parameter("fblob", [128, 512], F32, isOutput=False)
    wf1 = nc.declare_dram_parameter("wf1", [64, 16384], BF, isOutput=False)
    out_d = nc.declare_dram_parameter("out", [B, 9], F32, isOutput=True)

    rg = [list(range(ncores))]

    with tile.TileContext(nc) as tc:
        with (
            tc.tile_pool(name="persist", bufs=1) as pp,
            tc.tile_pool(name="xvp", bufs=4) as xvp,
            tc.tile_pool(name="small", bufs=1) as sp,
            tc.tile_pool(name="psA", bufs=5, space="PSUM") as psA,
            tc.tile_pool(name="psB", bufs=2, space="PSUM") as psB,
            tc.tile_pool(name="dram", bufs=1, space="DRAM") as dp,
        ):
            # ---- persistent SBUF ----
            hb1 = pp.tile([96, 32 * 341], BF, tag="hb1")
            hb2 = pp.tile([64, 32 * 100], BF, tag="hb2")
            hb3 = pp.tile([64, 32 * 64], BF, tag="hb3")
            t_all = pp.tile([64, 32 * 64], BF, tag="t_all")
            bfb = pp.tile([128, 1600], BF, tag="bfb")
            fbl = pp.tile([128, 512], F32, tag="fbl")
            wf1s = pp.tile([64, 16384], BF, tag="wf1s")
            w1x = bfb[0:56, 0:128]
            w1xB = bfb[64:120, 0:128]
            w2f = bfb[0:96, 128:640]
            w2f0 = bfb[0:32, 640:896]
            w3s = bfb[0:64, 896:1472]
            wf2s = bfb[:, 1472:1536]
            idbs = bfb[0:64, 1536:1600]
            rep3s = fbl[0:96, 0:32]
            bn1s = fbl[0:32, 32:34]
            bn2s = fbl[0:64, 34:36]
            bn3s = fbl[0:64, 36:38]
            bnf1s = fbl[:, 38:42]
            bnf2s = fbl[0:32, 42:44]
            wf3s = fbl[0:32, 44:53]
            b3s = fbl[0:1, 53:62]
            identf_s = fbl[0:32, 62:94]
            ones_s = fbl[0:1, 94:126]
            rep4 = fbl[:, 126:158]
            eye128 = fbl[:, 160:288]
            ones128 = fbl[0:1, 288:416]
            w2sc = pp.tile([96, 512], BF, tag="w2sc")
            w2sc0 = pp.tile([32, 256], BF, tag="w2sc0")
            w3sc = pp.tile([64, 576], BF, tag="w3sc")
            scrD = pp.tile([96, 682], BF, tag="scrD")
            scrA = pp.tile([96, 682], BF, tag="scrA")
            scrF = pp.tile([128, 64], F32, tag="scrF")
            s1pA = pp.tile([96, 24], F32, tag="s1pA")
            s1pD = pp.tile([96, 24], F32, tag="s1pD")
            s2pA = pp.tile([96, 24], F32, tag="s2pA")
            s2pD = pp.tile([96, 24], F32, tag="s2pD")
            junk = sp.tile([1, 48], F32, tag="junk")

            # ---- dummy AllReduce at t~0: absorbs mesh warmup + rank
            # skew on the CC queue while conv1 computes. Result unused;
            # kept live via the tail drain funnel.
            dar_i = dp.tile([1, 4], F32, tag="dari")
            dar_o = dp.tile([1, 4], F32, tag="daro")
            dummy_ar = nc.gpsimd.collective_compute(
                "AllReduce", ALU.add, replica_groups=rg,
                ins=[dar_i.opt()], outs=[dar_o.opt()])

            # ---- const loads ----
            nc.gpsimd.dma_start(out=bfb[:, :], in_=bfblob[:, :])
            nc.gpsimd.dma_start(out=fbl[:, :], in_=fblob[:, :])
            nc.gpsimd.tensor_copy(junk[0:1, 0:1], bfb[0:1, 0:2].bitcast(F32))
            nc.gpsimd.tensor_copy(junk[0:1, 1:2], fbl[0:1, 0:1])



            obsp = psB.tile([128, 16], F32, tag="obs", bufs=1)

            def pe_observe(ap, base=0):
                m = min(32, ap.shape[-1])
                return nc.tensor.matmul(
                    out=obsp[0:m, 0:1], lhsT=ap[..., 0:m], rhs=ap[..., 0:1],
                    start=True, stop=True, tile_position=(base, 0))

            # =============== conv1 ===============
            # 4 persistent input tiles (8 imgs each), groups 0,1 on
            # partitions 0:48, groups 2,3 on 64:112. Group DMAs chained so
            # arrivals pace the compute; no tile reuse -> no DMA hazards.
            hv = hb1.rearrange("p (i f) -> p i f", i=32)
            xqs = [pp.tile([128, 8 * 31 * 128], BF, tag=f"xq{g}",
                           name=f"xq{g}") for g in range(2)]
            grp_dmas = []
            for g in range(4):
                half = g % 2          # column half within the tile pair
                tilei = g // 2        # 0 -> partitions 0:48, 1 -> 64:112
                pb = 64 * tilei
                xq = xqs[half]
                xvv = xq.rearrange("k (i y x) -> k i y x", i=8, y=31)
                # K rows pb..pb+56: 24 raw + 8 zeros (one DMA) + 24
                # x-shifted (on-chip copy; both operands 32-aligned).
                d0 = nc.gpsimd.dma_start(
                    out=xq[pb:pb + 32, :],
                    in_=xin[32 * g:32 * g + 32, :])
                grp_dmas.append(d0)
                nc.gpsimd.tensor_copy(junk[0:1, 8 + 2 * g:9 + 2 * g],
                                      xq[pb:pb + 1, 0:2].bitcast(F32))
                # shifted copy: col 2j <- col 2j+1. The taps read the
                # shifted rows only at stride-4 offsets {0,2,4,6}+4k =
                # even columns, so odd dst columns are never read and
                # the copy moves half the bytes.
                nc.vector.tensor_copy(
                    xq[pb + 32:pb + 56, 0:31743:2],
                    xq[pb:pb + 24, 1:31744:2])
                if g == 0:
                    # burn the PE p-state ramp with ~30 free N=1 matmuls
                    for _ in range(30):
                        nc.tensor.matmul(
                            out=obsp[0:1, 0:1], lhsT=xq[0:24, 0:1],
                            rhs=xq[0:24, 0:1], start=True, stop=True)
                    pe_observe(w1x[0:48, 0:32])
                # absorb this group's DMA completion sem into PE
                # (single global SWDGE sem, so one observer covers both)
                # read only the copy-written rows: the copy already
                # waited on the DMA, so this LW carries ONE wait (TRN2
                # limit) and transitively covers the raw rows for the
                # real matmuls below.
                nc.tensor.matmul(out=obsp[0:1, 0:1],
                                 lhsT=xq[pb:pb + 24, 0:1],
                                 rhs=xq[pb:pb + 24, 0:1],
                                 start=True, stop=True,
                                 tile_position=(pb, 0))
                obs_mm = nc.tensor.matmul(out=obsp[0:1, 0:1],
                                          lhsT=xq[pb + 32:pb + 56, 0:1],
                                          rhs=xq[pb + 32:pb + 56, 0:1],
                                          start=True, stop=True,
                                          tile_position=(pb + 32, 0))
                wrow = w1x if tilei == 0 else w1xB
                # hb1 per-img layout is column-class grouped for conv2:
                # [cls0: yk-major 11x11 = 121][cls1: 11x10][cls2: 11x10]
                # (x' = 3j + cls). conv2 taps dx=1,2 then read one
                # contiguous 100-elem run per img (4 AP rows/matmul).
                for jj in range(8):
                    im = 8 * g + jj
                    bpair = im // 2
                    for c3 in range(3):
                        ny = C1_NY[c3]
                        nw = ny * 31
                        pt = psA.tile([128, 512], F32, tag="psA", name="c1pt")
                        for t in range(4):
                            mm = nc.tensor.matmul(
                                out=pt[32 * c3:32 * c3 + 32, 0:nw],
                                lhsT=wrow[:, 32 * t:32 * t + 32],
                                rhs=xvv[pb:pb + 56, jj:jj + 1, c3:
                                        c3 + 3 * (ny - 1) + 1:3,
                                        2 * t:2 * t + 121:4],
                                start=(t == 0), stop=(t == 3),
                                tile_position=(pb, 32 * c3))
                            if jj == 0 and c3 == 0 and t == 0:
                                add_dep_helper(mm.ins, obs_mm.ins,
                                               reason=f"dma-obs-{g}")
                        col = 2 * (bpair // 2) + (im % 2)
                        s1t = s1pA if bpair % 2 == 0 else s1pD
                        if bpair % 2 == 0:
                            nc.scalar.activation(
                                out=hv[32 * c3:32 * c3 + 32, im:im + 1, 0:nw],
                                in_=pt[32 * c3:32 * c3 + 32, 0:nw],
                                func=AF.Copy,
                                accum_out=s1t[32 * c3:32 * c3 + 32,
                                              col:col + 1])
                        else:
                            nc.vector.tensor_scalar(
                                out=hv[32 * c3:32 * c3 + 32, im:im + 1, 0:nw],
                                in0=pt[32 * c3:32 * c3 + 32, 0:nw],
                                scalar1=1.0, scalar2=None,
                                op0=ALU.mult, op1=ALU.add,
                                accum_out=s1t[32 * c3:32 * c3 + 32,
                                              col:col + 1])
                    if im % 2 == 1:
                        img0 = im - 1
                        gg = img0 // 4
                        s2t = s2pA if (im // 2) % 2 == 0 else s2pD
                        sct = scrA if (im // 2) % 2 == 0 else scrD
                        if (im // 2) % 2 == 0:
                            nc.scalar.activation(
                                out=sct[0:96, 0:620],
                                in_=hv[0:96, img0:img0 + 2, 0:310],
                                func=AF.Square,
                                accum_out=s2t[0:96, gg:gg + 1])
                            nc.scalar.activation(
                                out=sct[0:32, 620:682],
                                in_=hv[0:32, img0:img0 + 2, 310:341],
                                func=AF.Square,
                                accum_out=s2t[0:32, 8 + gg:9 + gg])
                        else:
                            nc.vector.scalar_tensor_tensor(
                                out=sct[0:96, 0:620],
                                in0=hv[0:96, img0:img0 + 2, 0:310],
                                scalar=1.0,
                                in1=hv[0:96, img0:img0 + 2, 0:310],
                                op0=ALU.mult, op1=ALU.mult,
                                accum_out=s2t[0:96, gg:gg + 1])
                            nc.vector.scalar_tensor_tensor(
                                out=sct[0:32, 620:682],
                                in0=hv[0:32, img0:img0 + 2, 310:341],
                                scalar=1.0,
                                in1=hv[0:32, img0:img0 + 2, 310:341],
                                op0=ALU.mult, op1=ALU.mult,
                                accum_out=s2t[0:32, 8 + gg:9 + gg])
            xv_dmas = grp_dmas

            # wf1 load after the xv stream (shares DMA bandwidth otherwise)
            wdma = nc.gpsimd.dma_start(out=wf1s[:, :], in_=wf1[:, :])
            add_dep_helper(wdma.ins, xv_dmas[-1].ins, reason="wf1-after-xv")
            nc.gpsimd.tensor_copy(junk[0:1, 2:3], wf1s[0:1, 0:2].bitcast(F32))

            # ---- bn1 ----
            pe_observe(rep3s[0:32, 0:32])
            t1 = sp.tile([96, 1], F32, tag="t1a")
            t2 = sp.tile([96, 1], F32, tag="t2a")
            tt1 = sp.tile([32, 1], F32, tag="tt1")
            tt2 = sp.tile([32, 1], F32, tag="tt2")
            ss1 = sp.tile([96, 2], F32, tag="ss1")
            for (sA, sD, col, tm, tt, m0, m1, u0, u1) in [
                    (s1pA, s1pD, 0, t1, tt1, 0, 16, None, None),
                    (s2pA, s2pD, 1, t2, tt2, 0, 8, 8, 16)]:
                nc.vector.tensor_reduce(out=tm[:, :], in_=sA[0:96, m0:m1],
                                        axis=mybir.AxisListType.X, op=ALU.add)
                nc.vector.tensor_copy(ss1[:, col:col + 1], tm[:, :])
                if u0 is not None:
                    nc.vector.tensor_reduce(out=tt[:, :], in_=sA[0:32, u0:u1],
                                            axis=mybir.AxisListType.X,
                                            op=ALU.add)
                    nc.vector.tensor_add(out=ss1[0:32, col:col + 1],
                                         in0=ss1[0:32, col:col + 1],
                                         in1=tt[:, :])
                nc.vector.tensor_reduce(out=tm[:, :], in_=sD[0:96, m0:m1],
                                        axis=mybir.AxisListType.X, op=ALU.add)
                nc.vector.tensor_add(out=ss1[0:96, col:col + 1],
                                     in0=ss1[0:96, col:col + 1], in1=tm[:, :])
                if u0 is not None:
                    nc.vector.tensor_reduce(out=tt[:, :], in_=sD[0:32, u0:u1],
                                            axis=mybir.AxisListType.X,
                                            op=ALU.add)
                    nc.vector.tensor_add(out=ss1[0:32, col:col + 1],
                                         in0=ss1[0:32, col:col + 1],
                                         in1=tt[:, :])
            ptb = psB.tile([128, 512], F32, tag="psB", name="bn1pt")
            nc.tensor.matmul(out=ptb[0:32, 0:2], lhsT=rep3s[0:96, 0:32],
                             rhs=ss1[:, :], start=True, stop=True)
            st1 = sp.tile([32, 4], F32, tag="st1")
            nc.scalar.copy(out=st1[:, 0:2], in_=ptb[0:32, 0:2])
            nc.scalar.copy(out=st1[:, 2:4], in_=bn1s[:, :])
            sc1, bp1 = _bn_allreduce(nc, sp, dp, st1, 32, N1, "ar1", junk, 40,
                                     rg, shift_mode="bias")
            sc96 = sp.tile([96, 1], F32, tag="sc96")
            bp96 = sp.tile([96, 1], F32, tag="bp96")
            for c3 in range(3):
                nc.scalar.copy(out=sc96[32 * c3:32 * c3 + 32, :], in_=sc1[:, :])
                nc.scalar.copy(out=bp96[32 * c3:32 * c3 + 32, :], in_=bp1[:, :])
            nc.scalar.mul(out=w2sc[:, :], in_=w2f[:, :], mul=sc96[:, 0:1])
            nc.scalar.mul(out=w2sc0[:, :], in_=w2f0[:, :], mul=sc96[0:32, 0:1])
            # relu1, bias-only, in 8 chunks of 4 imgs alternating ACT/DVE,
            # pipelined into conv2 below
            h1v = hb1.rearrange("p (i y x) -> p i y x", i=32, y=11)
            h2v = hb2.rearrange("p (i f) -> p i f", i=32)

            def relu1_chunk(ch):
                i0 = 4 * ch
                nc.vector.tensor_scalar(
                    out=hv[0:96, i0:i0 + 4, :], in0=hv[0:96, i0:i0 + 4, :],
                    scalar1=bp96[:, 0:1], scalar2=0.0,
                    op0=ALU.add, op1=ALU.max)

            # =============== conv2 ===============
            relu1_chunk(0)
            relu1_chunk(1)
            pe_observe(w2sc[0:32, 0:32])
            pe_observe(w2sc0[0:32, 0:32])
            pe_observe(hb1[0:48, 0:32])
            pe_observe(hb1[0:48, 4 * 341:4 * 341 + 32])
            # absorb the last conv1 ACT-eviction tick (psA bank WAR)
            pe_observe(s1pA[0:32, 0:16])
            obs_c2 = pe_observe(s2pA[0:32, 0:8])
            for ch in range(8):
                if ch + 2 < 8:
                    relu1_chunk(ch + 2)
                pt = psA.tile([128, 512], F32, tag="psA", name="c2pt")
                i0 = 4 * ch
                for dx in range(4):
                    mm2 = nc.tensor.matmul(
                        out=pt[0:64, 0:400],
                        lhsT=w2sc[0:96, 64 * dx:64 * dx + 64],
                        rhs=h1v[0:96, i0:i0 + 4, 0:10, dx:dx + 28:3],
                        start=(dx == 0), stop=False)
                    if ch == 0 and dx == 0:
                        add_dep_helper(mm2.ins, obs_c2.ins,
                                       reason="c2-obs-order")
                    nc.tensor.matmul(
                        out=pt[0:64, 0:400],
                        lhsT=w2sc0[0:32, 64 * dx:64 * dx + 64],
                        rhs=h1v[0:32, i0:i0 + 4, 1:11, dx:dx + 28:3],
                        start=False, stop=(dx == 3))
                g2 = ch // 2
                if ch % 2 == 0:
                    nc.scalar.activation(
                        out=h2v[0:64, i0:i0 + 4, :], in_=pt[0:64, 0:400],
                        func=AF.Copy, accum_out=s1pA[0:64, 16 + g2:17 + g2])
                    nc.scalar.activation(
                        out=scrA[0:64, 0:400],
                        in_=h2v[0:64, i0:i0 + 4, :], func=AF.Square,
                        accum_out=s2pA[0:64, 16 + g2:17 + g2])
                else:
                    nc.vector.tensor_scalar(
                        out=h2v[0:64, i0:i0 + 4, :], in0=pt[0:64, 0:400],
                        scalar1=1.0, scalar2=None, op0=ALU.mult, op1=ALU.add,
                        accum_out=s1pD[0:64, 16 + g2:17 + g2])
                    nc.vector.scalar_tensor_tensor(
                        out=scrD[0:64, 0:400],
                        in0=h2v[0:64, i0:i0 + 4, :], scalar=1.0,
                        in1=h2v[0:64, i0:i0 + 4, :],
                        op0=ALU.mult, op1=ALU.mult,
                        accum_out=s2pD[0:64, 16 + g2:17 + g2])

            # ---- bn2 ----
            st2 = sp.tile([64, 4], F32, tag="st2")
            ta = sp.tile([64, 1], F32, tag="t1b")
            tb = sp.tile([64, 1], F32, tag="t2b")
            ta2 = sp.tile([64, 1], F32, tag="t1b2")
            tb2 = sp.tile([64, 1], F32, tag="t2b2")
            nc.vector.tensor_reduce(out=ta[:, :], in_=s1pA[0:64, 16:20],
                                    axis=mybir.AxisListType.X, op=ALU.add)
            nc.vector.tensor_reduce(out=tb[:, :], in_=s1pD[0:64, 16:20],
                                    axis=mybir.AxisListType.X, op=ALU.add)
            nc.vector.tensor_add(out=st2[:, 0:1], in0=ta[:, :], in1=tb[:, :])
            nc.vector.tensor_reduce(out=ta2[:, :], in_=s2pA[0:64, 16:20],
                                    axis=mybir.AxisListType.X, op=ALU.add)
            nc.vector.tensor_reduce(out=tb2[:, :], in_=s2pD[0:64, 16:20],
                                    axis=mybir.AxisListType.X, op=ALU.add)
            nc.vector.tensor_add(out=st2[:, 1:2], in0=ta2[:, :], in1=tb2[:, :])
            nc.scalar.copy(out=st2[:, 2:4], in_=bn2s[:, :])
            sc2, bp2 = _bn_allreduce(nc, sp, dp, st2, 64, N2, "ar2", junk, 41,
                                     rg, shift_mode="bias")
            nc.scalar.mul(out=w3sc[:, :], in_=w3s[:, :], mul=sc2[:, 0:1])

            def relu2_chunk(ch):
                i0 = 8 * ch
                nc.vector.tensor_scalar(
                    out=h2v[0:64, i0:i0 + 8, :],
                    in0=h2v[0:64, i0:i0 + 8, :],
                    scalar1=bp2[:, 0:1], scalar2=0.0,
                    op0=ALU.add, op1=ALU.max)

            # =============== conv3 ===============
            relu2_chunk(0)
            relu2_chunk(1)
            pe_observe(w3sc[0:32, 0:32])
            pe_observe(hb2[0:32, 0:32])
            pe_observe(hb2[0:32, 8 * 100:8 * 100 + 32])
            # absorb conv2's ACT-eviction/scan ticks (psA bank WAR)
            pe_observe(s1pA[0:32, 16:20])
            obs_c3 = pe_observe(s2pA[0:32, 16:20])
            h2i = hb2.rearrange("p (i y x) -> p i y x", i=32, y=10)
            h3v = hb3.rearrange("p (i f) -> p i f", i=32)
            for ch in range(4):
                if ch + 2 < 4:
                    relu2_chunk(ch + 2)
                pt = psA.tile([128, 512], F32, tag="psA", name="c3pt")
                i0 = 8 * ch
                for dy in range(3):
                    for dx in range(3):
                        isplits = [(0, 8)]
                        for (is0, isn) in isplits:
                            mm3 = nc.tensor.matmul(
                                out=pt[0:64, 64 * is0:64 * (is0 + isn)],
                                lhsT=w3sc[0:64, 64 * (3 * dy + dx):
                                          64 * (3 * dy + dx) + 64],
                                rhs=h2i[0:64, i0 + is0:i0 + is0 + isn,
                                        dy:dy + 8, dx:dx + 8],
                                start=(dy == 0 and dx == 0),
                                stop=(dy == 2 and dx == 2))
                            if ch == 0 and dy == 0 and dx == 0:
                                add_dep_helper(mm3.ins, obs_c3.ins,
                                               reason="c3-obs-order")
                g3 = ch // 2
                if ch % 2 == 0:
                    nc.scalar.activation(
                        out=h3v[0:64, i0:i0 + 8, :], in_=pt[0:64, 0:512],
                        func=AF.Copy, accum_out=s1pA[0:64, 20 + g3:21 + g3])
                    nc.scalar.activation(
                        out=scrA[0:64, 0:512],
                        in_=h3v[0:64, i0:i0 + 8, :], func=AF.Square,
                        accum_out=s2pA[0:64, 20 + g3:21 + g3])
                else:
                    nc.vector.tensor_scalar(
                        out=h3v[0:64, i0:i0 + 8, :], in0=pt[0:64, 0:512],
                        scalar1=1.0, scalar2=None, op0=ALU.mult, op1=ALU.add,
                        accum_out=s1pD[0:64, 20 + g3:21 + g3])
                    nc.vector.scalar_tensor_tensor(
                        out=scrD[0:64, 0:512],
                        in0=h3v[0:64, i0:i0 + 8, :], scalar=1.0,
                        in1=h3v[0:64, i0:i0 + 8, :],
                        op0=ALU.mult, op1=ALU.mult,
                        accum_out=s2pD[0:64, 20 + g3:21 + g3])

            # ---- bn3 (standard scaled relu; fc1 follows) ----
            st3 = sp.tile([64, 4], F32, tag="st3")
            tc1 = sp.tile([64, 1], F32, tag="t1c")
            tc2 = sp.tile([64, 1], F32, tag="t2c")
            tc3 = sp.tile([64, 1], F32, tag="t1c2")
            tc4 = sp.tile([64, 1], F32, tag="t2c2")
            nc.vector.tensor_reduce(out=tc1[:, :], in_=s1pA[0:64, 20:22],
                                    axis=mybir.AxisListType.X, op=ALU.add)
            nc.vector.tensor_reduce(out=tc2[:, :], in_=s1pD[0:64, 20:22],
                                    axis=mybir.AxisListType.X, op=ALU.add)
            nc.vector.tensor_add(out=st3[:, 0:1], in0=tc1[:, :], in1=tc2[:, :])
            nc.vector.tensor_reduce(out=tc3[:, :], in_=s2pA[0:64, 20:22],
                                    axis=mybir.AxisListType.X, op=ALU.add)
            nc.vector.tensor_reduce(out=tc4[:, :], in_=s2pD[0:64, 20:22],
                                    axis=mybir.AxisListType.X, op=ALU.add)
            nc.vector.tensor_add(out=st3[:, 1:2], in0=tc3[:, :], in1=tc4[:, :])
            nc.scalar.copy(out=st3[:, 2:4], in_=bn3s[:, :])
            sc3, sh3 = _bn_allreduce(nc, sp, dp, st3, 64, N3, "ar3", junk, 42,
                                     rg, shift_mode="scaled")
            nc.scalar.activation(out=hb3[:, :], in_=hb3[:, :], func=AF.Relu,
                                 scale=sc3[:, 0:1], bias=sh3[:, 0:1])

            # =============== fc1 ===============
            pe_observe(sh3[0:32, 0:1])
            pe_observe(idbs[0:32, 0:32])
            for v in range(Bc):
                tp = psB.tile([64, 64], BF, tag="psB", name="tp")
                nc.tensor.transpose(
                    out=tp[0:64, 0:64],
                    in_=hb3[0:64, 64 * v:64 * v + 64],
                    identity=idbs[0:64, 0:64])
                nc.scalar.copy(out=t_all[0:64, 64 * v:64 * v + 64],
                               in_=tp[0:64, 0:64])

            pe_observe(wf1s[0:32, 0:32])
            tview = t_all.rearrange("p (v o) -> p o v", o=64)
            pf1s = sp.tile([128, 256], F32, tag="pf1s")
            for g in range(4):
                ptf1 = psB.tile([128, 512], F32, tag="psB", name="f1pt")
                for k in range(16):
                    oc = 16 * g + k
                    nc.tensor.matmul(
                        out=ptf1[32 * g:32 * g + 32, 0:256],
                        lhsT=tview[0:64, oc:oc + 1, :],
                        rhs=wf1s[:, 256 * oc:256 * oc + 256],
                        start=(k == 0), stop=(k == 15),
                        tile_position=(0, 32 * g))
                nc.scalar.copy(out=pf1s[32 * g:32 * g + 32, :],
                               in_=ptf1[32 * g:32 * g + 32, 0:256])
            ptf1b = psB.tile([128, 512], F32, tag="psB", name="f1ptb")
            nc.tensor.matmul(out=ptf1b[0:32, 0:256], lhsT=rep4[:, 0:32],
                             rhs=pf1s[:, :], start=True, stop=True)
            z4s = sp.tile([32, 256], F32, tag="z4s")
            nc.scalar.copy(out=z4s[:, :], in_=ptf1b[0:32, 0:256])

            # ======== fc tail: one AllGather of z4, then the whole
            # bnf1 -> fc2 -> bnf2 -> fc3 chain computed locally for the
            # FULL batch (identical on every core; out is [256, 9]).
            ag_i = dp.tile([32, 256], F32, tag="agi")
            ag_o = dp.tile([256, 256], F32, tag="ago")
            nc.gpsimd.dma_start(out=ag_i[:, :], in_=z4s[:, :])
            coll4 = nc.gpsimd.collective_compute(
                "AllGather", ALU.bypass, replica_groups=rg,
                ins=[ag_i.opt()], outs=[ag_o.opt()])
            # tail buffers alias dead xq0 columns (xq is input staging,
            # dead after conv1); bitcast for f32 views
            zgs = [xqs[0][0:128, 512 * u:512 * u + 512].bitcast(F32)
                   for u in range(2)]
            # absorb the PE lane into gpsimd first (t_all was written
            # by ACT after conv3's matmuls), so the zg DMAs' WAR on the
            # dead xq columns is covered and they carry ONE wait.
            nc.gpsimd.tensor_copy(junk[0:1, 45:46],
                                  t_all[0:1, 0:2].bitcast(F32))
            for u in range(2):
                nc.gpsimd.dma_start(
                    out=zgs[u][:, :], in_=ag_o[128 * u:128 * u + 128, :])
            pool_zg = nc.gpsimd.tensor_copy(junk[0:1, 43:44],
                                            zgs[1][0:1, 0:1])
            pe_observe(zgs[0][0:128, 0:32])
            pe_observe(zgs[1][0:128, 0:32])
            pe_observe(eye128[0:32, 0:32])
            # transpose the gathered [img 256, feat 256] to feat-major
            zT = [xqs[0][0:128, 1024 + 512 * f2:1536 + 512 * f2]
                  .bitcast(F32) for f2 in range(2)]
            for f2 in range(2):
                for u in range(2):
                    ztp = psB.tile([128, 512], F32, tag="psB", name="ztp")
                    nc.tensor.transpose(
                        out=ztp[0:128, 0:128],
                        in_=zgs[u][0:128, 128 * f2:128 * f2 + 128],
                        identity=eye128[:, :])
                    # all-ACT evictions keep every later zT consumer on
                    # a single sem lane
                    nc.scalar.copy(out=zT[f2][:, 128 * u:128 * u + 128],
                                   in_=ztp[0:128, 0:128])
            # bnf1: full-batch stats, computed locally (no collective);
            # all stat/relu ops on ACT, scratch in dead xq columns
            h4 = [xqs[0][0:128, 2048 + 256 * h:2304 + 256 * h]
                  for h in range(2)]
            stf = sp.tile([128, 8], F32, tag="stf")
            for f2 in range(2):
                nc.scalar.activation(
                    out=h4[0][:, :], in_=zT[f2][:, :], func=AF.Copy,
                    accum_out=stf[:, 4 * f2:4 * f2 + 1])
                nc.scalar.activation(
                    out=h4[1][:, :], in_=zT[f2][:, :], func=AF.Square,
                    accum_out=stf[:, 4 * f2 + 1:4 * f2 + 2])
                nc.scalar.copy(out=stf[:, 4 * f2 + 2:4 * f2 + 4],
                               in_=bnf1s[:, 2 * f2:2 * f2 + 2])
            sb4 = sp.tile([128, 4], F32, tag="sb4")
            sh4s = []
            for h in range(2):
                sc, sh, _ = _bn_scale_shift(nc, sp, stf[:, 4 * h:4 * h + 4],
                                            N4, 128, f"f1{h}")
                sh4s.append(sh)
                # stage scale/bias onto ACT so the relu has one lane
                nc.scalar.copy(out=sb4[:, 2 * h:2 * h + 1], in_=sc[:, :])
                nc.scalar.copy(out=sb4[:, 2 * h + 1:2 * h + 2], in_=sh[:, :])
                nc.scalar.activation(out=h4[h][:, :], in_=zT[h][:, :],
                                     func=AF.Relu,
                                     scale=sb4[:, 2 * h:2 * h + 1],
                                     bias=sb4[:, 2 * h + 1:2 * h + 2])

            # =============== fc2 (full batch) ===============
            pe_observe(sh4s[0][0:128, 0:1])
            pe_observe(sh4s[1][0:128, 0:1])
            pe_observe(wf2s[0:128, 0:32])
            ptf2 = psB.tile([128, 512], F32, tag="psB", name="f2pt")
            for h in range(2):
                nc.tensor.matmul(out=ptf2[0:32, 0:256],
                                 lhsT=wf2s[:, 32 * h:32 * h + 32],
                                 rhs=h4[h][:, :],
                                 start=(h == 0), stop=(h == 1))
            stf2 = sp.tile([32, 4], F32, tag="stf2")
            nc.scalar.activation(
                out=h4[0][0:32, :], in_=ptf2[0:32, 0:256], func=AF.Copy,
                accum_out=stf2[:, 0:1])
            nc.scalar.activation(
                out=h4[1][0:32, :], in_=ptf2[0:32, 0:256], func=AF.Square,
                accum_out=stf2[:, 1:2])
            nc.scalar.copy(out=stf2[:, 2:4], in_=bnf2s[:, :])
            sc5, sh5, dve_last = _bn_scale_shift(nc, sp, stf2, N5, 32, "f2")
            sb5 = sp.tile([32, 2], F32, tag="sb5")
            nc.scalar.copy(out=sb5[:, 0:1], in_=sc5[:, :])
            nc.scalar.copy(out=sb5[:, 1:2], in_=sh5[:, :])
            h5 = xqs[0][0:32, 2560:3072].bitcast(F32)
            nc.scalar.activation(out=h5[:, :], in_=ptf2[0:32, 0:256],
                                 func=AF.Relu,
                                 scale=sb5[:, 0:1], bias=sb5[:, 1:2])

            # =============== fc3 (full batch) ===============
            pe_observe(sh5[0:32, 0:1])
            pe_observe(wf3s[0:32, 0:4])
            pe_observe(ones128[0:1, 0:16])
            pe_observe(b3s[0:1, 0:4])
            outs = sp.tile([128, 18], F32, tag="outs")
            act_last = None
            pe_last = None
            out_dmas = []
            for u in range(2):
                ptf3 = psB.tile([128, 512], F32, tag="psB", name="f3pt")
                nc.tensor.matmul(out=ptf3[0:128, 0:9],
                                 lhsT=h5[:, 128 * u:128 * u + 128],
                                 rhs=wf3s[:, :], start=True, stop=False)
                pe_last = nc.tensor.matmul(out=ptf3[0:128, 0:9],
                                           lhsT=ones128[0:1, :],
                                           rhs=b3s[0:1, :],
                                           start=False, stop=True)
                act_last = nc.scalar.copy(out=outs[:, 9 * u:9 * u + 9],
                                          in_=ptf3[0:128, 0:9])
                out_dmas.append(nc.gpsimd.dma_start(
                    out=out_d[128 * u:128 * u + 128, :],
                    in_=outs[:, 9 * u:9 * u + 9]))

            for i, dep in enumerate([coll4, pool_zg, out_dmas[0],
                                     out_dmas[1], act_last, dve_last,
                                     pe_last, dummy_ar,
                                     grp_dmas[3], wdma]):
                dr = nc.sync.drain(fusable=False)
                add_dep_helper(dr.ins, dep.ins, reason=f"tail-funnel-{i}")

    # Strip same-engine PE sem waits from PE instructions: PE executes
    # in order, so a PE->PE WAW wait is redundant (the scheduler keeps
    # it when strided eviction reads don't provably cover a PSUM bank's
    # write range), and TRN2 allows only one sync wait per matmul.
    def _walk(b, out):
        out.append(b)
        for sb in getattr(b, 'blocks', []) or []:
            _walk(sb, out)
    blocks = []
    for f in nc.m.functions:
        for b in f.blocks:
            _walk(b, blocks)
    # per-engine record of sem values already waited on: an engine's
    # queue is in-order, so once it waited sem>=v every later
    # instruction on that engine inherits the guarantee
    waited = {}
    for b in blocks:
        for i in getattr(b, 'instructions', []) or []:
            si = getattr(i, 'sync_info', None)
            eng = str(getattr(i, 'engine', ''))
            if si and si.on_wait:
                w8 = waited.setdefault(eng, {})
                if len(si.on_wait) >= 2:
                    keep0 = [w for w in si.on_wait
                             if w8.get(w.ant_name, -1) < w.wait_value]
                    if keep0 and len(keep0) < len(si.on_wait):
                        si.on_wait = keep0
                for w in si.on_wait:
                    if w8.get(w.ant_name, -1) < w.wait_value:
                        w8[w.ant_name] = w.wait_value
            if si and len(si.on_wait or []) >= 2:
                own = str(getattr(i, 'engine', '')).split('.')[-1] + '_'
                keep = [w for w in si.on_wait
                        if not w.ant_name.startswith(own)]
                # a non-dummy collective completing implies conv1 (and
                # everything feeding its payload) is done, so engine
                # waits alongside a Collectives>=2 wait are dominated
                if len(keep) >= 2 and any(
                        w.ant_name.startswith('Collectives')
                        and w.wait_value >= 2 for w in keep):
                    keep = [w for w in keep
                            if w.ant_name.startswith('Collectives')]
                # two out-DMAs write disjoint out_d ranges; the coarse
                # whole-tensor WAW between them (a DMASW self-lane wait
                # on a DMA) is a false dependency
                if (len(keep) >= 2
                        and type(i).__name__ == 'InstDMACopy'):
                    nk = [w for w in keep
                          if not w.ant_name.startswith('DMASW')]
                    if nk:
                        keep = nk
                if keep and len(keep) < len(si.on_wait):
                    si.on_wait = keep
    # last resort: a Matmult with 2 remaining waits moves one onto its
    # preceding (same-engine, zero-wait) Ldweights — waiting earlier on
    # the in-order PE queue is always sound, and each TRN2 instruction
    # carries at most one wait
    for b in blocks:
        prev_lw = None
        for i in getattr(b, 'instructions', []) or []:
            tn = type(i).__name__
            si = getattr(i, 'sync_info', None)
            if tn == 'InstLdweights':
                prev_lw = i
                continue
            if (tn == 'InstMatmult' and si and len(si.on_wait or []) == 2
                    and prev_lw is not None):
                import bass_rust as _br
                lsi = getattr(prev_lw, 'sync_info', None)
                mv, kp = si.on_wait[0], si.on_wait[1]
                if lsi is None:
                    prev_lw.sync_info = _br.SyncInfo(on_wait=[mv],
                                                     on_update=[])
                    si.on_wait = [kp]
                elif not (lsi.on_wait or []):
                    lsi.on_wait = [mv]
                    si.on_wait = [kp]
            if tn == 'InstMatmult':
                prev_lw = None
    return nc


def _bn_scale_shift(nc, sp, gs, n, parts, name):
    """gs [parts,4] = (S1, S2, gamma, beta) -> (sc, shift, last_op),
    shift = beta - mean*sc."""
    m = sp.tile([parts, 1], F32, tag=f"m_{name}", name=f"m_{name}")
    q = sp.tile([parts, 1], F32, tag=f"q_{name}", name=f"q_{name}")
    msq = sp.tile([parts, 1], F32, tag=f"ms_{name}", name=f"ms_{name}")
    var = sp.tile([parts, 1], F32, tag=f"v_{name}", name=f"v_{name}")
    sd = sp.tile([parts, 1], F32, tag=f"sd_{name}", name=f"sd_{name}")
    rsd = sp.tile([parts, 1], F32, tag=f"rs_{name}", name=f"rs_{name}")
    sc = sp.tile([parts, 1], F32, tag=f"sc_{name}", name=f"sc_{name}")
    tmp = sp.tile([parts, 1], F32, tag=f"tp_{name}", name=f"tp_{name}")
    shf = sp.tile([parts, 1], F32, tag=f"sh_{name}", name=f"sh_{name}")
    obs = sp.tile([parts, 4], F32, tag=f"ob_{name}", name=f"ob_{name}")
    nc.vector.tensor_copy(obs[:, :], gs[:, :])
    nc.scalar.mul(out=m[:, :], in_=gs[:, 0:1], mul=1.0 / n)
    nc.scalar.mul(out=q[:, :], in_=gs[:, 1:2], mul=1.0 / n)
    nc.scalar.square(out=msq[:, :], in_=m[:, :])
    nc.vector.tensor_sub(out=var[:, :], in0=q[:, :], in1=msq[:, :])
    nc.vector.tensor_scalar_add(out=var[:, :], in0=var[:, :], scalar1=EPS)
    nc.scalar.sqrt(out=sd[:, :], in_=var[:, :])
    nc.vector.reciprocal(out=rsd[:, :], in_=sd[:, :])
    nc.vector.tensor_mul(out=sc[:, :], in0=rsd[:, :], in1=gs[:, 2:3])
    nc.vector.tensor_mul(out=tmp[:, :], in0=m[:, :], in1=sc[:, :])
    last = nc.vector.tensor_sub(out=shf[:, :], in0=gs[:, 3:4], in1=tmp[:, :])
    return sc, shf, last


def _bn_allreduce(nc, sp, dp, st, parts, n, name, junk_g, jcol, rg,
                  shift_mode="bias"):
    """DMA st -> AllReduce -> gs; compute (sc, b).

    "bias": b = beta*sd/gamma - mean (unscaled relu; fold sc into the next
    layer's weights).  "scaled": b = beta - mean*sc (standard form).
    """
    arin = dp.tile([parts, 4], F32, tag=f"{name}i", name=f"{name}i")
    arout = dp.tile([parts, 4], F32, tag=f"{name}o", name=f"{name}o")
    nc.gpsimd.dma_start(out=arin[:, :], in_=st[:, :])
    nc.gpsimd.collective_compute(
        "AllReduce", ALU.add, replica_groups=rg,
        ins=[arin.opt()], outs=[arout.opt()])
    gs = sp.tile([parts, 4], F32, tag=f"gs_{name}", name=f"gs_{name}")
    nc.gpsimd.dma_start(out=gs[:, :], in_=arout[:, :])
    nc.gpsimd.tensor_copy(junk_g[0:1, jcol:jcol + 1], gs[0:1, 0:1])
    if shift_mode == "scaled":
        sc, shf, _ = _bn_scale_shift(nc, sp, gs, n, parts, name)
        return sc, shf
    m = sp.tile([parts, 1], F32, tag=f"m_{name}", name=f"m_{name}")
    q = sp.tile([parts, 1], F32, tag=f"q_{name}", name=f"q_{name}")
    msq = sp.tile([parts, 1], F32, tag=f"ms_{name}", name=f"ms_{name}")
    var = sp.tile([parts, 1], F32, tag=f"v_{name}", name=f"v_{name}")
    sd = sp.tile([parts, 1], F32, tag=f"sd_{name}", name=f"sd_{name}")
    rsd = sp.tile([parts, 1], F32, tag=f"rs_{name}", name=f"rs_{name}")
    sc = sp.tile([parts, 1], F32, tag=f"sc_{name}", name=f"sc_{name}")
    rg_ = sp.tile([parts, 1], F32, tag=f"rg_{name}", name=f"rg_{name}")
    tb_ = sp.tile([parts, 1], F32, tag=f"tb_{name}", name=f"tb_{name}")
    bp = sp.tile([parts, 1], F32, tag=f"bp_{name}", name=f"bp_{name}")
    obs = sp.tile([parts, 4], F32, tag=f"ob_{name}", name=f"ob_{name}")
    nc.vector.tensor_copy(obs[:, :], gs[:, :])
    nc.scalar.mul(out=m[:, :], in_=gs[:, 0:1], mul=1.0 / n)
    nc.scalar.mul(out=q[:, :], in_=gs[:, 1:2], mul=1.0 / n)
    nc.scalar.square(out=msq[:, :], in_=m[:, :])
    nc.vector.tensor_sub(out=var[:, :], in0=q[:, :], in1=msq[:, :])
    nc.vector.tensor_scalar_add(out=var[:, :], in0=var[:, :], scalar1=EPS)
    nc.scalar.sqrt(out=sd[:, :], in_=var[:, :])
    nc.vector.reciprocal(out=rsd[:, :], in_=sd[:, :])
    nc.vector.tensor_mul(out=sc[:, :], in0=rsd[:, :], in1=gs[:, 2:3])
    nc.vector.reciprocal(out=rg_[:, :], in_=gs[:, 2:3])
    nc.vector.tensor_mul(out=tb_[:, :], in0=gs[:, 3:4], in1=sd[:, :])
    nc.vector.tensor_mul(out=tb_[:, :], in0=tb_[:, :], in1=rg_[:, :])
    nc.vector.tensor_sub(out=bp[:, :], in0=tb_[:, :], in1=m[:, :])
    return sc, bp


# ---------------------------------------------------------------------------
def _prep_consts(conv1_w, conv2_w, conv3_w, fc1_w, fc2_w, fc3_w, fc3_b,
                 bn1_g, bn1_b, bn2_g, bn2_b, bn3_g, bn3_b,
                 bnf1_g, bnf1_b, bnf2_g, bnf2_b, ncores=NCORES):
    sgn = lambda w: np.sign(np.asarray(w)).astype(np.float32)
    w1, w2, w3 = sgn(conv1_w), sgn(conv2_w), sgn(conv3_w)
    wf1_, wf2_ = sgn(fc1_w), sgn(fc2_w)

    # SBUF K rows: 0:24 = raw (xc=0), 24:32 = zero spacer,
    # 32:56 = x-shifted copy (xc=1)
    w1x = np.zeros((56, 128), np.float32)
    for xc in range(2):
        for c in range(3):
            for dy in range(8):
                for t in range(4):
                    w1x[32 * xc + c * 8 + dy, 32 * t:32 * t + 32] = \
                        w1[:, c, dy, xc + 2 * t]
    w2f = np.zeros((96, 512), np.float32)
    for c3 in range(3):
        for dx in range(4):
            w2f[32 * c3:32 * c3 + 32, 64 * dx:64 * dx + 64] = w2[:, :, c3, dx].T
    w2f0 = np.zeros((32, 256), np.float32)
    for dx in range(4):
        w2f0[:, 64 * dx:64 * dx + 64] = w2[:, :, 3, dx].T
    w3r = np.ascontiguousarray(w3.transpose(1, 2, 3, 0).reshape(64, 9 * 64))
    wf1r = np.ascontiguousarray(
        wf1_.reshape(256, 64, 64).transpose(2, 1, 0).reshape(64, 64 * 256))
    wf2r = np.ascontiguousarray(
        wf2_.reshape(32, 2, 128).transpose(2, 1, 0).reshape(128, 64))
    wf3r = np.ascontiguousarray(np.asarray(fc3_w).astype(np.float32).T)
    b3r = np.asarray(fc3_b).astype(np.float32).reshape(1, 9)

    bfblob = np.zeros((128, 1600), np.float32)
    bfblob[0:56, 0:128] = w1x
    bfblob[64:120, 0:128] = w1x
    bfblob[0:96, 128:640] = w2f
    bfblob[0:32, 640:896] = w2f0
    bfblob[0:64, 896:1472] = w3r
    bfblob[:, 1472:1536] = wf2r
    bfblob[0:64, 1536:1600] = np.eye(64, dtype=np.float32)

    rep = lambda g, b: np.stack(
        [np.asarray(g), np.asarray(b)], axis=1).astype(np.float32) / ncores
    fblob = np.zeros((128, 512), np.float32)
    fblob[0:96, 0:32] = np.tile(np.eye(32, dtype=np.float32), (3, 1))
    fblob[0:32, 32:34] = rep(bn1_g, bn1_b)
    fblob[0:64, 34:36] = rep(bn2_g, bn2_b)
    fblob[0:64, 36:38] = rep(bn3_g, bn3_b)
    fblob[:, 38:42] = np.concatenate([
        np.stack([np.asarray(bnf1_g)[:128], np.asarray(bnf1_b)[:128]], 1),
        np.stack([np.asarray(bnf1_g)[128:], np.asarray(bnf1_b)[128:]], 1)],
        axis=1).astype(np.float32)
    fblob[0:32, 42:44] = np.stack(
        [np.asarray(bnf2_g), np.asarray(bnf2_b)], axis=1).astype(np.float32)
    fblob[0:32, 44:53] = wf3r
    fblob[0:1, 53:62] = b3r
    fblob[0:32, 62:94] = np.eye(32, dtype=np.float32)
    fblob[0:1, 94:126] = 1.0
    fblob[:, 126:158] = np.tile(np.eye(32, dtype=np.float32), (4, 1))
    fblob[:, 160:288] = np.eye(128, dtype=np.float32)
    fblob[0:1, 288:416] = 1.0
    return {
        "bfblob": bfblob.astype(bf16),
        "fblob": fblob,
        "wf1": wf1r.astype(bf16),
    }


def _prep_xr(xc):
    """-> xin [128, 8*31*128]: rows 32g..32g+32 = group g of 8 imgs:
    24 xr k-rows + 8 zero spacer rows, free laid out (img, y, x) so
    each group DMA is one contiguous-per-partition transfer."""
    out = np.empty((xc.shape[0], 24, 31, 128), dtype=bf16)
    for c in range(3):
        for dy in range(8):
            out[:, c * 8 + dy] = xc[:, c, dy:dy + 121:4, :].astype(bf16)
    xin = np.zeros((128, 8 * 31 * 128), dtype=bf16)
    for g in range(4):
        blk = out[8 * g:8 * g + 8].transpose(1, 0, 2, 3)   # [24,8,31,128]
        xin[32 * g:32 * g + 24] = blk.reshape(24, -1)
    return xin


_NC_CACHE = None


def kernel(**inputs):
    global _NC_CACHE
    if _NC_CACHE is None:
        _NC_CACHE = build_program()
    nc = _NC_CACHE

    x = np.asarray(inputs["x"])
    consts = _prep_consts(
        inputs["conv1_w"], inputs["conv2_w"], inputs["conv3_w"],
        inputs["fc1_w"], inputs["fc2_w"], inputs["fc3_w"], inputs["fc3_b"],
        inputs["bn1_g"], inputs["bn1_b"], inputs["bn2_g"], inputs["bn2_b"],
        inputs["bn3_g"], inputs["bn3_b"],
        inputs["bnf1_g"], inputs["bnf1_b"], inputs["bnf2_g"], inputs["bnf2_b"])

    in_maps = []
    for i in range(NCORES):
        m = dict(consts)
        m["xin"] = _prep_xr(x[Bc * i:Bc * (i + 1)])
        in_maps.append(m)

    res = run_bass_kernel_spmd(nc, in_maps, list(range(NCORES)))
    return np.asarray(res.results[0]["out"]).astype(np.float32)


if __name__ == "__main__":
    nc = build_program()
    print("program built ok")



# revision 77
# speedup vs baseline: 1.0071x; 1.0071x over previous
"""BinarizedConvNet forward on 8 Trainium2 cores, v3.

Measured-hardware facts this version targets:
  - PE matmul cost ~ (free_size x pe_cycle) + ~12-15ns per rhs AP row;
    LDWEIGHTS overlaps on its own pipe; clock ramps with continuous use.
  - DMA: ~8 GB/s per channel x 16 channels ~ 127 GB/s aggregate,
    regardless of descriptor size or HW/SW DGE. Input bytes are the
    conv1 floor, so only the raw x rows ship (6.1MB + 2MB zero spacer);
    the x-shifted copy for odd-dx taps is built on-chip by DVE at
    stride 2 (only even columns of the shifted rows are ever read).
  - First collective pays ~90us mesh warmup: a dummy AllReduce issued
    at t~0 absorbs it (and most rank-start skew) under conv1.
  - TRN2 instructions carry at most ONE semaphore wait; a post-pass
    strips same-engine and dominated waits and relocates a second
    matmul wait onto its preceding LDWEIGHTS.

Structure:
  conv1: K=56 rows per group = 24 raw + 8 zero spacer + 24 x-shifted
    (engine ops need 32-aligned partition starts; the zero weights over
    the spacer keep PE's IEEE 0*NaN poison away). 4 groups of 8 imgs,
    4 taps of K=56 per (img, rowclass), outputs class-packed at psum
    partitions 32*c3.
  conv2: per dx one K=96 MM (dy 0,1,2) + one K=32 MM (dy=3): 8 MMs per
    4-img chunk; relu1 is bias-only on DVE (bn1 scale folded into w2).
  conv3: 36 dense K=64 MMs, 8 imgs per chunk; relu2 on DVE.
  fc1: PE transposes + oc-group matmuls + indicator-sum matmul.
  fc tail: ONE AllGather of the fc1 pre-activations (z4), then
    bnf1 -> fc2 -> bnf2 -> fc3 computed for the FULL batch on every
    core (output [256, 9], identical across cores; buffers alias dead
    xq columns).

bn1/bn2/bn3 use exact global batch stats via AllReduce of (sum, sumsq)
with gamma/beta pre-divided by NCORES; bnf1/bnf2 are local after the
AllGather (gamma/beta undivided).
"""

import numpy as np
import ml_dtypes

import concourse.bass as bass
import concourse.mybir as mybir
import concourse.tile as tile
import concourse.tile_sem_assignment as _tsa
from concourse.tile_rust import add_dep_helper
from concourse.bass_utils import run_bass_kernel_spmd

_tsa.NUM_SWDGE_GLOBAL_SEMS = 4

dt = mybir.dt
BF, F32 = dt.bfloat16, dt.float32
AF = mybir.ActivationFunctionType
ALU = mybir.AluOpType
bf16 = ml_dtypes.bfloat16

NCORES = 8
Bc = 32
EPS = 1e-5
B = 256

N1 = B * 31 * 31
N2 = B * 10 * 10
N3 = B * 8 * 8
N4 = B
N5 = B

C1_NY = [11, 10, 10]   # conv1 class row counts (y' = c3 + 3k)


def build_program(ncores=NCORES):
    nc = bass.Bass(num_swdge_queues=4)

    # xin rows 32g..32g+32 = group g (8 imgs): 24 xr k-rows + 8 zero
    # spacer rows, free = (img 8, y 31, x 128) contiguous per partition.
    # The x-shifted copy (odd-dx taps) is built on-chip: DMA BW
    # (~127 GB/s aggregate) is the conv1 floor, so shrinking DRAM bytes
    # wins even at the cost of an on-chip 24-lane copy per group. The
    # zero spacer keeps every SBUF K row initialized (no NaN garbage
    # under the zero weight rows) while keeping engine-op partition
    # starts 32-aligned.
    xin = nc.declare_dram_parameter("xin", [128, 8 * 31 * 128], BF,
                                    isOutput=False)
    bfblob = nc.declare_dram_parameter("bfblob", [128, 1600], BF, isOutput=False)
    fblob = nc.declare_dram_parameter("fblob", [128, 512], F32, isOutput=False)
    wf1 = nc.declare_dram_parameter("wf1", [64, 16384], BF, isOutput=False)
    out_d = nc.declare_dram_parameter("out", [B, 9], F32, isOutput=True)

    rg = [list(range(ncores))]

    with tile.TileContext(nc) as tc:
        with (
            tc.tile_pool(name="persist", bufs=1) as pp,
            tc.tile_pool(name="xvp", bufs=4) as xvp,
            tc.tile_pool(name="small", bufs=1) as sp,
            tc.tile_pool(name="psA", bufs=5, space="PSUM") as psA,
            tc.tile_pool(name="psB", bufs=2, space="PSUM") as psB,
            tc.tile_pool(name="dram", bufs=1, space="DRAM") as dp,
        ):
            # ---- persistent SBUF ----
            hb1 = pp.tile([96, 32 * 341], BF, tag="hb1")
            hb2 = pp.tile([64, 32 * 100], BF, tag="hb2")
            hb3 = pp.tile([64, 32 * 64], BF, tag="hb3")
            t_all = pp.tile([64, 32 * 64], BF, tag="t_all")
            bfb = pp.tile([128, 1600], BF, tag="bfb")
            fbl = pp.tile([128, 512], F32, tag="fbl")
            wf1s = pp.tile([64, 16384], BF, tag="wf1s")
            w1x = bfb[0:56, 0:128]
            w1xB = bfb[64:120, 0:128]
            w2f = bfb[0:96, 128:640]
            w2f0 = bfb[0:32, 640:896]
            w3s = bfb[0:64, 896:1472]
            wf2s = bfb[:, 1472:1536]
            idbs = bfb[0:64, 1536:1600]
            rep3s = fbl[0:96, 0:32]
            bn1s = fbl[0:32, 32:34]
            bn2s = fbl[0:64, 34:36]
            bn3s = fbl[0:64, 36:38]
            bnf1s = fbl[:, 38:42]
            bnf2s = fbl[0:32, 42:44]
            wf3s = fbl[0:32, 44:53]
            b3s = fbl[0:1, 53:62]
            identf_s = fbl[0:32, 62:94]
            ones_s = fbl[0:1, 94:126]
            rep4 = fbl[:, 126:158]
            eye128 = fbl[:, 160:288]
            ones128 = fbl[0:1, 288:416]
            w2sc = pp.tile([96, 512], BF, tag="w2sc")
            w2sc0 = pp.tile([32, 256], BF, tag="w2sc0")
            w3sc = pp.tile([64, 576], BF, tag="w3sc")
            scrD = pp.tile([96, 682], BF, tag="scrD")
            scrA = pp.tile([96, 682], BF, tag="scrA")
            scrF = pp.tile([128, 64], F32, tag="scrF")
            s1pA = pp.tile([96, 24], F32, tag="s1pA")
            s1pD = pp.tile([96, 24], F32, tag="s1pD")
            s2pA = pp.tile([96, 24], F32, tag="s2pA")
            s2pD = pp.tile([96, 24], F32, tag="s2pD")
            junk = sp.tile([1, 48], F32, tag="junk")

            # ---- dummy AllReduce at t~0: absorbs mesh warmup + rank
            # skew on the CC queue while conv1 computes. Result unused;
            # kept live via the tail drain funnel.
            dar_i = dp.tile([1, 4], F32, tag="dari")
            dar_o = dp.tile([1, 4], F32, tag="daro")
            dummy_ar = nc.gpsimd.collective_compute(
                "AllReduce", ALU.add, replica_groups=rg,
                ins=[dar_i.opt()], outs=[dar_o.opt()])

            # ---- const loads ----
            nc.gpsimd.dma_start(out=bfb[:, :], in_=bfblob[:, :])
            nc.gpsimd.dma_start(out=fbl[:, :], in_=fblob[:, :])
            nc.gpsimd.tensor_copy(junk[0:1, 0:1], bfb[0:1, 0:2].bitcast(F32))
            nc.gpsimd.tensor_copy(junk[0:1, 1:2], fbl[0:1, 0:1])



            obsp = psB.tile([128, 16], F32, tag="obs", bufs=1)

            def pe_observe(ap, base=0):
                m = min(32, ap.shape[-1])
                return nc.tensor.matmul(
                    out=obsp[0:m, 0:1], lhsT=ap[..., 0:m], rhs=ap[..., 0:1],
                    start=True, stop=True, tile_position=(base, 0))

            # =============== conv1 ===============
            # 4 persistent input tiles (8 imgs each), groups 0,1 on
            # partitions 0:48, groups 2,3 on 64:112. Group DMAs chained so
            # arrivals pace the compute; no tile reuse -> no DMA hazards.
            hv = hb1.rearrange("p (i f) -> p i f", i=32)
            xqs = [pp.tile([128, 8 * 31 * 128], BF, tag=f"xq{g}",
                           name=f"xq{g}") for g in range(2)]
            grp_dmas = []
            for g in range(4):
                half = g % 2          # column half within the tile pair
                tilei = g // 2        # 0 -> partitions 0:48, 1 -> 64:112
                pb = 64 * tilei
                xq = xqs[half]
                xvv = xq.rearrange("k (i y x) -> k i y x", i=8, y=31)
                # K rows pb..pb+56: 24 raw + 8 zeros (one DMA) + 24
                # x-shifted (on-chip copy; both operands 32-aligned).
                d0 = nc.gpsimd.dma_start(
                    out=xq[pb:pb + 32, :],
                    in_=xin[32 * g:32 * g + 32, :])
                grp_dmas.append(d0)
                nc.gpsimd.tensor_copy(junk[0:1, 8 + 2 * g:9 + 2 * g],
                                      xq[pb:pb + 1, 0:2].bitcast(F32))
                # shifted copy: col 2j <- col 2j+1. The taps read the
                # shifted rows only at stride-4 offsets {0,2,4,6}+4k =
                # even columns, so odd dst columns are never read and
                # the copy moves half the bytes.
                nc.vector.tensor_copy(
                    xq[pb + 32:pb + 56, 0:31743:2],
                    xq[pb:pb + 24, 1:31744:2])
                if g == 0:
                    # burn the PE p-state ramp with ~30 free N=1 matmuls
                    for _ in range(30):
                        nc.tensor.matmul(
                            out=obsp[0:1, 0:1], lhsT=xq[0:24, 0:1],
                            rhs=xq[0:24, 0:1], start=True, stop=True)
                    pe_observe(w1x[0:48, 0:32])
                # absorb this group's DMA completion sem into PE
                # (single global SWDGE sem, so one observer covers both)
                # read only the copy-written rows: the copy already
                # waited on the DMA, so this LW carries ONE wait (TRN2
                # limit) and transitively covers the raw rows for the
                # real matmuls below.
                nc.tensor.matmul(out=obsp[0:1, 0:1],
                                 lhsT=xq[pb:pb + 24, 0:1],
                                 rhs=xq[pb:pb + 24, 0:1],
                                 start=True, stop=True,
                                 tile_position=(pb, 0))
                obs_mm = nc.tensor.matmul(out=obsp[0:1, 0:1],
                                          lhsT=xq[pb + 32:pb + 56, 0:1],
                                          rhs=xq[pb + 32:pb + 56, 0:1],
                                          start=True, stop=True,
                                          tile_position=(pb + 32, 0))
                wrow = w1x if tilei == 0 else w1xB
                # hb1 per-img layout is column-class grouped for conv2:
                # [cls0: yk-major 11x11 = 121][cls1: 11x10][cls2: 11x10]
                # (x' = 3j + cls). conv2 taps dx=1,2 then read one
                # contiguous 100-elem run per img (4 AP rows/matmul).
                for jj in range(8):
                    im = 8 * g + jj
                    bpair = im // 2
                    for c3 in range(3):
                        ny = C1_NY[c3]
                        nw = ny * 31
                        pt = psA.tile([128, 512], F32, tag="psA", name="c1pt")
                        for t in range(4):
                            mm = nc.tensor.matmul(
                                out=pt[32 * c3:32 * c3 + 32, 0:nw],
                                lhsT=wrow[:, 32 * t:32 * t + 32],
                                rhs=xvv[pb:pb + 56, jj:jj + 1, c3:
                                        c3 + 3 * (ny - 1) + 1:3,
                                        2 * t:2 * t + 121:4],
                                start=(t == 0), stop=(t == 3),
                                tile_position=(pb, 32 * c3))
                            if jj == 0 and c3 == 0 and t == 0:
                                add_dep_helper(mm.ins, obs_mm.ins,
                                               reason=f"dma-obs-{g}")
                        col = 2 * (bpair // 2) + (im % 2)
                        s1t = s1pA if bpair % 2 == 0 else s1pD
                        if bpair % 2 == 0:
                            nc.scalar.activation(
                                out=hv[32 * c3:32 * c3 + 32, im:im + 1, 0:nw],
                                in_=pt[32 * c3:32 * c3 + 32, 0:nw],
                                func=AF.Copy,
                                accum_out=s1t[32 * c3:32 * c3 + 32,
                                              col:col + 1])
                        else:
                            nc.vector.tensor_scalar(
                                out=hv[32 * c3:32 * c3 + 32, im:im + 1, 0:nw],
                                in0=pt[32 * c3:32 * c3 + 32, 0:nw],
                                scalar1=1.0, scalar2=None,
                                op0=ALU.mult, op1=ALU.add,
                                accum_out=s1t[32 * c3:32 * c3 + 32,
                                              col:col + 1])
                    if im % 2 == 1:
                        img0 = im - 1
                        gg = img0 // 4
                        s2t = s2pA if (im // 2) % 2 == 0 else s2pD
                        sct = scrA if (im // 2) % 2 == 0 else scrD
                        if (im // 2) % 2 == 0:
                            nc.scalar.activation(
                                out=sct[0:96, 0:620],
                                in_=hv[0:96, img0:img0 + 2, 0:310],
                                func=AF.Square,
                                accum_out=s2t[0:96, gg:gg + 1])
                            nc.scalar.activation(
                                out=sct[0:32, 620:682],
                                in_=hv[0:32, img0:img0 + 2, 310:341],
                                func=AF.Square,
                                accum_out=s2t[0:32, 8 + gg:9 + gg])
                        else:
                            nc.vector.scalar_tensor_tensor(
                                out=sct[0:96, 0:620],
                                in0=hv[0:96, img0:img0 + 2, 0:310],
                                scalar=1.0,
                                in1=hv[0:96, img0:img0 + 2, 0:310],
                                op0=ALU.mult, op1=ALU.mult,
                                accum_out=s2t[0:96, gg:gg + 1])
                            nc.vector.scalar_tensor_tensor(
                                out=sct[0:32, 620:682],
                                in0=hv[0:32, img0:img0 + 2, 310:341],
                                scalar=1.0,
                                in1=hv[0:32, img0:img0 + 2, 310:341],
                                op0=ALU.mult, op1=ALU.mult,
                                accum_out=s2t[0:32, 8 + gg:9 + gg])
            xv_dmas = grp_dmas

            # wf1 load after the xv stream (shares DMA bandwidth otherwise)
            wdma = nc.gpsimd.dma_start(out=wf1s[:, :], in_=wf1[:, :])
            add_dep_helper(wdma.ins, xv_dmas[-1].ins, reason="wf1-after-xv")
            nc.gpsimd.tensor_copy(junk[0:1, 2:3], wf1s[0:1, 0:2].bitcast(F32))

            # ---- bn1 ----
            pe_observe(rep3s[0:32, 0:32])
            t1 = sp.tile([96, 1], F32, tag="t1a")
            t2 = sp.tile([96, 1], F32, tag="t2a")
            tt1 = sp.tile([32, 1], F32, tag="tt1")
            tt2 = sp.tile([32, 1], F32, tag="tt2")
            ss1 = sp.tile([96, 2], F32, tag="ss1")
            for (sA, sD, col, tm, tt, m0, m1, u0, u1) in [
                    (s1pA, s1pD, 0, t1, tt1, 0, 16, None, None),
                    (s2pA, s2pD, 1, t2, tt2, 0, 8, 8, 16)]:
                nc.vector.tensor_reduce(out=tm[:, :], in_=sA[0:96, m0:m1],
                                        axis=mybir.AxisListType.X, op=ALU.add)
                nc.vector.tensor_copy(ss1[:, col:col + 1], tm[:, :])
                if u0 is not None:
                    nc.vector.tensor_reduce(out=tt[:, :], in_=sA[0:32, u0:u1],
                                            axis=mybir.AxisListType.X,
                                            op=ALU.add)
                    nc.vector.tensor_add(out=ss1[0:32, col:col + 1],
                                         in0=ss1[0:32, col:col + 1],
                                         in1=tt[:, :])
                nc.vector.tensor_reduce(out=tm[:, :], in_=sD[0:96, m0:m1],
                                        axis=mybir.AxisListType.X, op=ALU.add)
                nc.vector.tensor_add(out=ss1[0:96, col:col + 1],
                                     in0=ss1[0:96, col:col + 1], in1=tm[:, :])
                if u0 is not None:
                    nc.vector.tensor_reduce(out=tt[:, :], in_=sD[0:32, u0:u1],
                                            axis=mybir.AxisListType.X,
                                            op=ALU.add)
                    nc.vector.tensor_add(out=ss1[0:32, col:col + 1],
                                         in0=ss1[0:32, col:col + 1],
                                         in1=tt[:, :])
            ptb = psB.tile([128, 512], F32, tag="psB", name="bn1pt")
            nc.tensor.matmul(out=ptb[0:32, 0:2], lhsT=rep3s[0:96, 0:32],
                             rhs=ss1[:, :], start=True, stop=True)
            st1 = sp.tile([32, 4], F32, tag="st1")
            nc.scalar.copy(out=st1[:, 0:2], in_=ptb[0:32, 0:2])
            nc.scalar.copy(out=st1[:, 2:4], in_=bn1s[:, :])
            sc1, bp1 = _bn_allreduce(nc, sp, dp, st1, 32, N1, "ar1", junk, 40,
                                     rg, shift_mode="bias")
            sc96 = sp.tile([96, 1], F32, tag="sc96")
            bp96 = sp.tile([96, 1], F32, tag="bp96")
            for c3 in range(3):
                nc.scalar.copy(out=sc96[32 * c3:32 * c3 + 32, :], in_=sc1[:, :])
                nc.scalar.copy(out=bp96[32 * c3:32 * c3 + 32, :], in_=bp1[:, :])
            nc.scalar.mul(out=w2sc[:, :], in_=w2f[:, :], mul=sc96[:, 0:1])
            nc.scalar.mul(out=w2sc0[:, :], in_=w2f0[:, :], mul=sc96[0:32, 0:1])
            # relu1, bias-only, in 8 chunks of 4 imgs alternating ACT/DVE,
            # pipelined into conv2 below
            h1v = hb1.rearrange("p (i y x) -> p i y x", i=32, y=11)
            h2v = hb2.rearrange("p (i f) -> p i f", i=32)

            def relu1_chunk(ch):
                i0 = 4 * ch
                nc.vector.tensor_scalar(
                    out=hv[0:96, i0:i0 + 4, :], in0=hv[0:96, i0:i0 + 4, :],
                    scalar1=bp96[:, 0:1], scalar2=0.0,
                    op0=ALU.add, op1=ALU.max)

            # =============== conv2 ===============
            relu1_chunk(0)
            relu1_chunk(1)
            pe_observe(w2sc[0:32, 0:32])
            pe_observe(w2sc0[0:32, 0:32])
            pe_observe(hb1[0:48, 0:32])
            pe_observe(hb1[0:48, 4 * 341:4 * 341 + 32])
            # absorb the last conv1 ACT-eviction tick (psA bank WAR)
            pe_observe(s1pA[0:32, 0:16])
            obs_c2 = pe_observe(s2pA[0:32, 0:8])
            for ch in range(8):
                if ch + 2 < 8:
                    relu1_chunk(ch + 2)
                pt = psA.tile([128, 512], F32, tag="psA", name="c2pt")
                i0 = 4 * ch
                for dx in range(4):
                    mm2 = nc.tensor.matmul(
                        out=pt[0:64, 0:400],
                        lhsT=w2sc[0:96, 64 * dx:64 * dx + 64],
                        rhs=h1v[0:96, i0:i0 + 4, 0:10, dx:dx + 28:3],
                        start=(dx == 0), stop=False)
                    if ch == 0 and dx == 0:
                        add_dep_helper(mm2.ins, obs_c2.ins,
                                       reason="c2-obs-order")
                    nc.tensor.matmul(
                        out=pt[0:64, 0:400],
                        lhsT=w2sc0[0:32, 64 * dx:64 * dx + 64],
                        rhs=h1v[0:32, i0:i0 + 4, 1:11, dx:dx + 28:3],
                        start=False, stop=(dx == 3))
                g2 = ch // 2
                if ch % 2 == 0:
                    nc.scalar.activation(
                        out=h2v[0:64, i0:i0 + 4, :], in_=pt[0:64, 0:400],
                        func=AF.Copy, accum_out=s1pA[0:64, 16 + g2:17 + g2])
                    nc.scalar.activation(
                        out=scrA[0:64, 0:400],
                        in_=h2v[0:64, i0:i0 + 4, :], func=AF.Square,
                        accum_out=s2pA[0:64, 16 + g2:17 + g2])
                else:
                    nc.vector.tensor_scalar(
                        out=h2v[0:64, i0:i0 + 4, :], in0=pt[0:64, 0:400],
                        scalar1=1.0, scalar2=None, op0=ALU.mult, op1=ALU.add,
                        accum_out=s1pD[0:64, 16 + g2:17 + g2])
                    nc.vector.scalar_tensor_tensor(
                        out=scrD[0:64, 0:400],
                        in0=h2v[0:64, i0:i0 + 4, :], scalar=1.0,
                        in1=h2v[0:64, i0:i0 + 4, :],
                        op0=ALU.mult, op1=ALU.mult,
                        accum_out=s2pD[0:64, 16 + g2:17 + g2])

            # ---- bn2 ----
            st2 = sp.tile([64, 4], F32, tag="st2")
            ta = sp.tile([64, 1], F32, tag="t1b")
            tb = sp.tile([64, 1], F32, tag="t2b")
            ta2 = sp.tile([64, 1], F32, tag="t1b2")
            tb2 = sp.tile([64, 1], F32, tag="t2b2")
            nc.vector.tensor_reduce(out=ta[:, :], in_=s1pA[0:64, 16:20],
                                    axis=mybir.AxisListType.X, op=ALU.add)
            nc.vector.tensor_reduce(out=tb[:, :], in_=s1pD[0:64, 16:20],
                                    axis=mybir.AxisListType.X, op=ALU.add)
            nc.vector.tensor_add(out=st2[:, 0:1], in0=ta[:, :], in1=tb[:, :])
            nc.vector.tensor_reduce(out=ta2[:, :], in_=s2pA[0:64, 16:20],
                                    axis=mybir.AxisListType.X, op=ALU.add)
            nc.vector.tensor_reduce(out=tb2[:, :], in_=s2pD[0:64, 16:20],
                                    axis=mybir.AxisListType.X, op=ALU.add)
            nc.vector.tensor_add(out=st2[:, 1:2], in0=ta2[:, :], in1=tb2[:, :])
            nc.scalar.copy(out=st2[:, 2:4], in_=bn2s[:, :])
            sc2, bp2 = _bn_allreduce(nc, sp, dp, st2, 64, N2, "ar2", junk, 41,
                                     rg, shift_mode="bias")
            nc.scalar.mul(out=w3sc[:, :], in_=w3s[:, :], mul=sc2[:, 0:1])

            def relu2_chunk(ch):
                i0 = 8 * ch
                nc.vector.tensor_scalar(
                    out=h2v[0:64, i0:i0 + 8, :],
                    in0=h2v[0:64, i0:i0 + 8, :],
                    scalar1=bp2[:, 0:1], scalar2=0.0,
                    op0=ALU.add, op1=ALU.max)

            # =============== conv3 ===============
            relu2_chunk(0)
            relu2_chunk(1)
            pe_observe(w3sc[0:32, 0:32])
            pe_observe(hb2[0:32, 0:32])
            pe_observe(hb2[0:32, 8 * 100:8 * 100 + 32])
            # absorb conv2's ACT-eviction/scan ticks (psA bank WAR)
            pe_observe(s1pA[0:32, 16:20])
            obs_c3 = pe_observe(s2pA[0:32, 16:20])
            h2i = hb2.rearrange("p (i y x) -> p i y x", i=32, y=10)
            h3v = hb3.rearrange("p (i f) -> p i f", i=32)
            for ch in range(4):
                if ch + 2 < 4:
                    relu2_chunk(ch + 2)
                pt = psA.tile([128, 512], F32, tag="psA", name="c3pt")
                i0 = 8 * ch
                for dy in range(3):
                    for dx in range(3):
                        isplits = [(0, 8)]
                        for (is0, isn) in isplits:
                            mm3 = nc.tensor.matmul(
                                out=pt[0:64, 64 * is0:64 * (is0 + isn)],
                                lhsT=w3sc[0:64, 64 * (3 * dy + dx):
                                          64 * (3 * dy + dx) + 64],
                                rhs=h2i[0:64, i0 + is0:i0 + is0 + isn,
                                        dy:dy + 8, dx:dx + 8],
                                start=(dy == 0 and dx == 0),
                                stop=(dy == 2 and dx == 2))
                            if ch == 0 and dy == 0 and dx == 0:
                                add_dep_helper(mm3.ins, obs_c3.ins,
                                               reason="c3-obs-order")
                g3 = ch // 2
                if ch % 2 == 0:
                    nc.scalar.activation(
                        out=h3v[0:64, i0:i0 + 8, :], in_=pt[0:64, 0:512],
                        func=AF.Copy, accum_out=s1pA[0:64, 20 + g3:21 + g3])
                    nc.scalar.activation(
                        out=scrA[0:64, 0:512],
                        in_=h3v[0:64, i0:i0 + 8, :], func=AF.Square,
                        accum_out=s2pA[0:64, 20 + g3:21 + g3])
                else:
                    nc.vector.tensor_scalar(
                        out=h3v[0:64, i0:i0 + 8, :], in0=pt[0:64, 0:512],
                        scalar1=1.0, scalar2=None, op0=ALU.mult, op1=ALU.add,
                        accum_out=s1pD[0:64, 20 + g3:21 + g3])
                    nc.vector.scalar_tensor_tensor(
                        out=scrD[0:64, 0:512],
                        in0=h3v[0:64, i0:i0 + 8, :], scalar=1.0,
                        in1=h3v[0:64, i0:i0 + 8, :],
                        op0=ALU.mult, op1=ALU.mult,
                        accum_out=s2pD[0:64, 20 + g3:21 + g3])

            # ---- bn3 (standard scaled relu; fc1 follows) ----
            st3 = sp.tile([64, 4], F32, tag="st3")
            tc1 = sp.tile([64, 1], F32, tag="t1c")
            tc2 = sp.tile([64, 1], F32, tag="t2c")
            tc3 = sp.tile([64, 1], F32, tag="t1c2")
            tc4 = sp.tile([64, 1], F32, tag="t2c2")
            nc.vector.tensor_reduce(out=tc1[:, :], in_=s1pA[0:64, 20:22],
                                    axis=mybir.AxisListType.X, op=ALU.add)
            nc.vector.tensor_reduce(out=tc2[:, :], in_=s1pD[0:64, 20:22],
                                    axis=mybir.AxisListType.X, op=ALU.add)
            nc.vector.tensor_add(out=st3[:, 0:1], in0=tc1[:, :], in1=tc2[:, :])
            nc.vector.tensor_reduce(out=tc3[:, :], in_=s2pA[0:64, 20:22],
                                    axis=mybir.AxisListType.X, op=ALU.add)
            nc.vector.tensor_reduce(out=tc4[:, :], in_=s2pD[0:64, 20:22],
                                    axis=mybir.AxisListType.X, op=ALU.add)
            nc.vector.tensor_add(out=st3[:, 1:2], in0=tc3[:, :], in1=tc4[:, :])
            nc.scalar.copy(out=st3[:, 2:4], in_=bn3s[:, :])
            sc3, sh3 = _bn_allreduce(nc, sp, dp, st3, 64, N3, "ar3", junk, 42,
                                     rg, shift_mode="scaled")
            nc.scalar.activation(out=hb3[:, :], in_=hb3[:, :], func=AF.Relu,
                                 scale=sc3[:, 0:1], bias=sh3[:, 0:1])

            # =============== fc1 ===============
            pe_observe(sh3[0:32, 0:1])
            pe_observe(idbs[0:32, 0:32])
            for v in range(Bc):
                tp = psB.tile([64, 64], BF, tag="psB", name="tp")
                nc.tensor.transpose(
                    out=tp[0:64, 0:64],
                    in_=hb3[0:64, 64 * v:64 * v + 64],
                    identity=idbs[0:64, 0:64])
                nc.scalar.copy(out=t_all[0:64, 64 * v:64 * v + 64],
                               in_=tp[0:64, 0:64])

            pe_observe(wf1s[0:32, 0:32])
            tview = t_all.rearrange("p (v o) -> p o v", o=64)
            pf1s = sp.tile([128, 256], F32, tag="pf1s")
            for g in range(4):
                ptf1 = psB.tile([128, 512], F32, tag="psB", name="f1pt")
                for k in range(16):
                    oc = 16 * g + k
                    nc.tensor.matmul(
                        out=ptf1[32 * g:32 * g + 32, 0:256],
                        lhsT=tview[0:64, oc:oc + 1, :],
                        rhs=wf1s[:, 256 * oc:256 * oc + 256],
                        start=(k == 0), stop=(k == 15),
                        tile_position=(0, 32 * g))
                nc.scalar.copy(out=pf1s[32 * g:32 * g + 32, :],
                               in_=ptf1[32 * g:32 * g + 32, 0:256])
            ptf1b = psB.tile([128, 512], F32, tag="psB", name="f1ptb")
            nc.tensor.matmul(out=ptf1b[0:32, 0:256], lhsT=rep4[:, 0:32],
                             rhs=pf1s[:, :], start=True, stop=True)
            z4s = sp.tile([32, 256], F32, tag="z4s")
            nc.scalar.copy(out=z4s[:, :], in_=ptf1b[0:32, 0:256])

            # ======== fc tail: one AllGather of z4, then the whole
            # bnf1 -> fc2 -> bnf2 -> fc3 chain computed locally for the
            # FULL batch (identical on every core; out is [256, 9]).
            ag_i = dp.tile([32, 256], F32, tag="agi")
            ag_o = dp.tile([256, 256], F32, tag="ago")
            nc.gpsimd.dma_start(out=ag_i[:, :], in_=z4s[:, :])
            coll4 = nc.gpsimd.collective_compute(
                "AllGather", ALU.bypass, replica_groups=rg,
                ins=[ag_i.opt()], outs=[ag_o.opt()])
            # tail buffers alias dead xq0 columns (xq is input staging,
            # dead after conv1); bitcast for f32 views
            zgs = [xqs[0][0:128, 512 * u:512 * u + 512].bitcast(F32)
                   for u in range(2)]
            # absorb the PE lane into gpsimd first (t_all was written
            # by ACT after conv3's matmuls), so the zg DMAs' WAR on the
            # dead xq columns is covered and they carry ONE wait.
            nc.gpsimd.tensor_copy(junk[0:1, 45:46],
                                  t_all[0:1, 0:2].bitcast(F32))
            zg_dmas = []
            for u in range(2):
                zg_dmas.append(nc.gpsimd.dma_start(
                    out=zgs[u][:, :], in_=ag_o[128 * u:128 * u + 128, :]))
            pool_zg = nc.gpsimd.tensor_copy(junk[0:1, 43:44],
                                            zgs[1][0:1, 0:1])
            pe_observe(zgs[0][0:128, 0:32])
            pe_observe(zgs[1][0:128, 0:32])
            pe_observe(eye128[0:32, 0:32])
            # transpose the gathered [img 256, feat 256] to feat-major
            zT = [xqs[0][0:128, 1024 + 512 * f2:1536 + 512 * f2]
                  .bitcast(F32) for f2 in range(2)]
            for f2 in range(2):
                for u in range(2):
                    ztp = psB.tile([128, 512], F32, tag="psB", name="ztp")
                    nc.tensor.transpose(
                        out=ztp[0:128, 0:128],
                        in_=zgs[u][0:128, 128 * f2:128 * f2 + 128],
                        identity=eye128[:, :])
                    # all-ACT evictions keep every later zT consumer on
                    # a single sem lane
                    nc.scalar.copy(out=zT[f2][:, 128 * u:128 * u + 128],
                                   in_=ztp[0:128, 0:128])
            # bnf1: full-batch stats, computed locally (no collective);
            # all stat/relu ops on ACT, scratch in dead xq columns
            h4 = [xqs[0][0:128, 2048 + 256 * h:2304 + 256 * h]
                  for h in range(2)]
            stf = sp.tile([128, 8], F32, tag="stf")
            for f2 in range(2):
                nc.scalar.activation(
                    out=h4[0][:, :], in_=zT[f2][:, :], func=AF.Copy,
                    accum_out=stf[:, 4 * f2:4 * f2 + 1])
                nc.scalar.activation(
                    out=h4[1][:, :], in_=zT[f2][:, :], func=AF.Square,
                    accum_out=stf[:, 4 * f2 + 1:4 * f2 + 2])
                nc.scalar.copy(out=stf[:, 4 * f2 + 2:4 * f2 + 4],
                               in_=bnf1s[:, 2 * f2:2 * f2 + 2])
            sb4 = sp.tile([128, 4], F32, tag="sb4")
            sh4s = []
            for h in range(2):
                sc, sh, _ = _bn_scale_shift(nc, sp, stf[:, 4 * h:4 * h + 4],
                                            N4, 128, f"f1{h}")
                sh4s.append(sh)
                # stage scale/bias onto ACT so the relu has one lane
                nc.scalar.copy(out=sb4[:, 2 * h:2 * h + 1], in_=sc[:, :])
                nc.scalar.copy(out=sb4[:, 2 * h + 1:2 * h + 2], in_=sh[:, :])
                nc.scalar.activation(out=h4[h][:, :], in_=zT[h][:, :],
                                     func=AF.Relu,
                                     scale=sb4[:, 2 * h:2 * h + 1],
                                     bias=sb4[:, 2 * h + 1:2 * h + 2])

            # =============== fc2 (full batch) ===============
            pe_observe(sh4s[0][0:128, 0:1])
            pe_observe(sh4s[1][0:128, 0:1])
            pe_observe(wf2s[0:128, 0:32])
            ptf2 = psB.tile([128, 512], F32, tag="psB", name="f2pt")
            for h in range(2):
                nc.tensor.matmul(out=ptf2[0:32, 0:256],
                                 lhsT=wf2s[:, 32 * h:32 * h + 32],
                                 rhs=h4[h][:, :],
                                 start=(h == 0), stop=(h == 1))
            stf2 = sp.tile([32, 4], F32, tag="stf2")
            nc.scalar.activation(
                out=h4[0][0:32, :], in_=ptf2[0:32, 0:256], func=AF.Copy,
                accum_out=stf2[:, 0:1])
            nc.scalar.activation(
                out=h4[1][0:32, :], in_=ptf2[0:32, 0:256], func=AF.Square,
                accum_out=stf2[:, 1:2])
            nc.scalar.copy(out=stf2[:, 2:4], in_=bnf2s[:, :])
            sc5, sh5, dve_last = _bn_scale_shift(nc, sp, stf2, N5, 32, "f2")
            sb5 = sp.tile([32, 2], F32, tag="sb5")
            nc.scalar.copy(out=sb5[:, 0:1], in_=sc5[:, :])
            nc.scalar.copy(out=sb5[:, 1:2], in_=sh5[:, :])
            h5 = xqs[0][0:32, 2560:3072].bitcast(F32)
            nc.scalar.activation(out=h5[:, :], in_=ptf2[0:32, 0:256],
                                 func=AF.Relu,
                                 scale=sb5[:, 0:1], bias=sb5[:, 1:2])

            # =============== fc3 (full batch) ===============
            pe_observe(sh5[0:32, 0:1])
            pe_observe(wf3s[0:32, 0:4])
            pe_observe(ones128[0:1, 0:16])
            pe_observe(b3s[0:1, 0:4])
            outs = sp.tile([128, 18], F32, tag="outs")
            act_last = None
            pe_last = None
            out_dmas = []
            for u in range(2):
                ptf3 = psB.tile([128, 512], F32, tag="psB", name="f3pt")
                nc.tensor.matmul(out=ptf3[0:128, 0:9],
                                 lhsT=h5[:, 128 * u:128 * u + 128],
                                 rhs=wf3s[:, :], start=True, stop=False)
                pe_last = nc.tensor.matmul(out=ptf3[0:128, 0:9],
                                           lhsT=ones128[0:1, :],
                                           rhs=b3s[0:1, :],
                                           start=False, stop=True)
                act_last = nc.scalar.copy(out=outs[:, 9 * u:9 * u + 9],
                                          in_=ptf3[0:128, 0:9])
                out_dmas.append(nc.gpsimd.dma_start(
                    out=out_d[128 * u:128 * u + 128, :],
                    in_=outs[:, 9 * u:9 * u + 9]))

            for i, dep in enumerate([coll4, pool_zg, out_dmas[0],
                                     out_dmas[1], act_last, dve_last,
                                     pe_last, dummy_ar, zg_dmas[0],
                                     zg_dmas[1], grp_dmas[3], wdma]):
                dr = nc.sync.drain(fusable=False)
                add_dep_helper(dr.ins, dep.ins, reason=f"tail-funnel-{i}")

    # Strip same-engine PE sem waits from PE instructions: PE executes
    # in order, so a PE->PE WAW wait is redundant (the scheduler keeps
    # it when strided eviction reads don't provably cover a PSUM bank's
    # write range), and TRN2 allows only one sync wait per matmul.
    def _walk(b, out):
        out.append(b)
        for sb in getattr(b, 'blocks', []) or []:
            _walk(sb, out)
    blocks = []
    for f in nc.m.functions:
        for b in f.blocks:
            _walk(b, blocks)
    # per-engine record of sem values already waited on: an engine's
    # queue is in-order, so once it waited sem>=v every later
    # instruction on that engine inherits the guarantee
    waited = {}
    for b in blocks:
        for i in getattr(b, 'instructions', []) or []:
            si = getattr(i, 'sync_info', None)
            eng = str(getattr(i, 'engine', ''))
            if si and si.on_wait:
                w8 = waited.setdefault(eng, {})
                if len(si.on_wait) >= 2:
                    keep0 = [w for w in si.on_wait
                             if w8.get(w.ant_name, -1) < w.wait_value]
                    if keep0 and len(keep0) < len(si.on_wait):
                        si.on_wait = keep0
                for w in si.on_wait:
                    if w8.get(w.ant_name, -1) < w.wait_value:
                        w8[w.ant_name] = w.wait_value
            if si and len(si.on_wait or []) >= 2:
                own = str(getattr(i, 'engine', '')).split('.')[-1] + '_'
                keep = [w for w in si.on_wait
                        if not w.ant_name.startswith(own)]
                # a non-dummy collective completing implies conv1 (and
                # everything feeding its payload) is done, so engine
                # waits alongside a Collectives>=2 wait are dominated
                if len(keep) >= 2 and any(
                        w.ant_name.startswith('Collectives')
                        and w.wait_value >= 2 for w in keep):
                    keep = [w for w in keep
                            if w.ant_name.startswith('Collectives')]
                # two out-DMAs write disjoint out_d ranges; the coarse
                # whole-tensor WAW between them (a DMASW self-lane wait
                # on a DMA) is a false dependency
                if (len(keep) >= 2
                        and type(i).__name__ == 'InstDMACopy'):
                    nk = [w for w in keep
                          if not w.ant_name.startswith('DMASW')]
                    if nk:
                        keep = nk
                if keep and len(keep) < len(si.on_wait):
                    si.on_wait = keep
    # last resort: a Matmult with 2 remaining waits moves one onto its
    # preceding (same-engine, zero-wait) Ldweights — waiting earlier on
    # the in-order PE queue is always sound, and each TRN2 instruction
    # carries at most one wait
    for b in blocks:
        prev_lw = None
        for i in getattr(b, 'instructions', []) or []:
            tn = type(i).__name__
            si = getattr(i, 'sync_info', None)
            if tn == 'InstLdweights':
                prev_lw = i
                continue
            if (tn == 'InstMatmult' and si and len(si.on_wait or []) == 2
                    and prev_lw is not None):
                import bass_rust as _br
                lsi = getattr(prev_lw, 'sync_info', None)
                mv, kp = si.on_wait[0], si.on_wait[1]
                if lsi is None:
                    prev_lw.sync_info = _br.SyncInfo(on_wait=[mv],
                                                     on_update=[])
                    si.on_wait = [kp]
                elif not (lsi.on_wait or []):
                    lsi.on_wait = [mv]
                    si.on_wait = [kp]
            if tn == 'InstMatmult':
                prev_lw = None
    return nc


def _bn_scale_shift(nc, sp, gs, n, parts, name):
    """gs [parts,4] = (S1, S2, gamma, beta) -> (sc, shift, last_op),
    shift = beta - mean*sc."""
    m = sp.tile([parts, 1], F32, tag=f"m_{name}", name=f"m_{name}")
    q = sp.tile([parts, 1], F32, tag=f"q_{name}", name=f"q_{name}")
    msq = sp.tile([parts, 1], F32, tag=f"ms_{name}", name=f"ms_{name}")
    var = sp.tile([parts, 1], F32, tag=f"v_{name}", name=f"v_{name}")
    sd = sp.tile([parts, 1], F32, tag=f"sd_{name}", name=f"sd_{name}")
    rsd = sp.tile([parts, 1], F32, tag=f"rs_{name}", name=f"rs_{name}")
    sc = sp.tile([parts, 1], F32, tag=f"sc_{name}", name=f"sc_{name}")
    tmp = sp.tile([parts, 1], F32, tag=f"tp_{name}", name=f"tp_{name}")
    shf = sp.tile([parts, 1], F32, tag=f"sh_{name}", name=f"sh_{name}")
    obs = sp.tile([parts, 4], F32, tag=f"ob_{name}", name=f"ob_{name}")
    nc.vector.tensor_copy(obs[:, :], gs[:, :])
    nc.scalar.mul(out=m[:, :], in_=gs[:, 0:1], mul=1.0 / n)
    nc.scalar.mul(out=q[:, :], in_=gs[:, 1:2], mul=1.0 / n)
    nc.scalar.square(out=msq[:, :], in_=m[:, :])
    nc.vector.tensor_sub(out=var[:, :], in0=q[:, :], in1=msq[:, :])
    nc.vector.tensor_scalar_add(out=var[:, :], in0=var[:, :], scalar1=EPS)
    nc.scalar.sqrt(out=sd[:, :], in_=var[:, :])
    nc.vector.reciprocal(out=rsd[:, :], in_=sd[:, :])
    nc.vector.tensor_mul(out=sc[:, :], in0=rsd[:, :], in1=gs[:, 2:3])
    nc.vector.tensor_mul(out=tmp[:, :], in0=m[:, :], in1=sc[:, :])
    last = nc.vector.tensor_sub(out=shf[:, :], in0=gs[:, 3:4], in1=tmp[:, :])
    return sc, shf, last


def _bn_allreduce(nc, sp, dp, st, parts, n, name, junk_g, jcol, rg,
                  shift_mode="bias"):
    """DMA st -> AllReduce -> gs; compute (sc, b).

    "bias": b = beta*sd/gamma - mean (unscaled relu; fold sc into the next
    layer's weights).  "scaled": b = beta - mean*sc (standard form).
    """
    arin = dp.tile([parts, 4], F32, tag=f"{name}i", name=f"{name}i")
    arout = dp.tile([parts, 4], F32, tag=f"{name}o", name=f"{name}o")
    nc.gpsimd.dma_start(out=arin[:, :], in_=st[:, :])
    nc.gpsimd.collective_compute(
        "AllReduce", ALU.add, replica_groups=rg,
        ins=[arin.opt()], outs=[arout.opt()])
    gs = sp.tile([parts, 4], F32, tag=f"gs_{name}", name=f"gs_{name}")
    nc.gpsimd.dma_start(out=gs[:, :], in_=arout[:, :])
    nc.gpsimd.tensor_copy(junk_g[0:1, jcol:jcol + 1], gs[0:1, 0:1])
    if shift_mode == "scaled":
        sc, shf, _ = _bn_scale_shift(nc, sp, gs, n, parts, name)
        return sc, shf
    m = sp.tile([parts, 1], F32, tag=f"m_{name}", name=f"m_{name}")
    q = sp.tile([parts, 1], F32, tag=f"q_{name}", name=f"q_{name}")
    msq = sp.tile([parts, 1], F32, tag=f"ms_{name}", name=f"ms_{name}")
    var = sp.tile([parts, 1], F32, tag=f"v_{name}", name=f"v_{name}")
    sd = sp.tile([parts, 1], F32, tag=f"sd_{name}", name=f"sd_{name}")
    rsd = sp.tile([parts, 1], F32, tag=f"rs_{name}", name=f"rs_{name}")
    sc = sp.tile([parts, 1], F32, tag=f"sc_{name}", name=f"sc_{name}")
    rg_ = sp.tile([parts, 1], F32, tag=f"rg_{name}", name=f"rg_{name}")
    tb_ = sp.tile([parts, 1], F32, tag=f"tb_{name}", name=f"tb_{name}")
    bp = sp.tile([parts, 1], F32, tag=f"bp_{name}", name=f"bp_{name}")
    obs = sp.tile([parts, 4], F32, tag=f"ob_{name}", name=f"ob_{name}")
    nc.vector.tensor_copy(obs[:, :], gs[:, :])
    nc.scalar.mul(out=m[:, :], in_=gs[:, 0:1], mul=1.0 / n)
    nc.scalar.mul(out=q[:, :], in_=gs[:, 1:2], mul=1.0 / n)
    nc.scalar.square(out=msq[:, :], in_=m[:, :])
    nc.vector.tensor_sub(out=var[:, :], in0=q[:, :], in1=msq[:, :])
    nc.vector.tensor_scalar_add(out=var[:, :], in0=var[:, :], scalar1=EPS)
    nc.scalar.sqrt(out=sd[:, :], in_=var[:, :])
    nc.vector.reciprocal(out=rsd[:, :], in_=sd[:, :])
    nc.vector.tensor_mul(out=sc[:, :], in0=rsd[:, :], in1=gs[:, 2:3])
    nc.vector.reciprocal(out=rg_[:, :], in_=gs[:, 2:3])
    nc.vector.tensor_mul(out=tb_[:, :], in0=gs[:, 3:4], in1=sd[:, :])
    nc.vector.tensor_mul(out=tb_[:, :], in0=tb_[:, :], in1=rg_[:, :])
    nc.vector.tensor_sub(out=bp[:, :], in0=tb_[:, :], in1=m[:, :])
    return sc, bp


# ---------------------------------------------------------------------------
def _prep_consts(conv1_w, conv2_w, conv3_w, fc1_w, fc2_w, fc3_w, fc3_b,
                 bn1_g, bn1_b, bn2_g, bn2_b, bn3_g, bn3_b,
                 bnf1_g, bnf1_b, bnf2_g, bnf2_b, ncores=NCORES):
    sgn = lambda w: np.sign(np.asarray(w)).astype(np.float32)
    w1, w2, w3 = sgn(conv1_w), sgn(conv2_w), sgn(conv3_w)
    wf1_, wf2_ = sgn(fc1_w), sgn(fc2_w)

    # SBUF K rows: 0:24 = raw (xc=0), 24:32 = zero spacer,
    # 32:56 = x-shifted copy (xc=1)
    w1x = np.zeros((56, 128), np.float32)
    for xc in range(2):
        for c in range(3):
            for dy in range(8):
                for t in range(4):
                    w1x[32 * xc + c * 8 + dy, 32 * t:32 * t + 32] = \
                        w1[:, c, dy, xc + 2 * t]
    w2f = np.zeros((96, 512), np.float32)
    for c3 in range(3):
        for dx in range(4):
            w2f[32 * c3:32 * c3 + 32, 64 * dx:64 * dx + 64] = w2[:, :, c3, dx].T
    w2f0 = np.zeros((32, 256), np.float32)
    for dx in range(4):
        w2f0[:, 64 * dx:64 * dx + 64] = w2[:, :, 3, dx].T
    w3r = np.ascontiguousarray(w3.transpose(1, 2, 3, 0).reshape(64, 9 * 64))
    wf1r = np.ascontiguousarray(
        wf1_.reshape(256, 64, 64).transpose(2, 1, 0).reshape(64, 64 * 256))
    wf2r = np.ascontiguousarray(
        wf2_.reshape(32, 2, 128).transpose(2, 1, 0).reshape(128, 64))
    wf3r = np.ascontiguousarray(np.asarray(fc3_w).astype(np.float32).T)
    b3r = np.asarray(fc3_b).astype(np.float32).reshape(1, 9)

    bfblob = np.zeros((128, 1600), np.float32)
    bfblob[0:56, 0:128] = w1x
    bfblob[64:120, 0:128] = w1x
    bfblob[0:96, 128:640] = w2f
    bfblob[0:32, 640:896] = w2f0
    bfblob[0:64, 896:1472] = w3r
    bfblob[:, 1472:1536] = wf2r
    bfblob[0:64, 1536:1600] = np.eye(64, dtype=np.float32)

    rep = lambda g, b: np.stack(
        [np.asarray(g), np.asarray(b)], axis=1).astype(np.float32) / ncores
    fblob = np.zeros((128, 512), np.float32)
    fblob[0:96, 0:32] = np.tile(np.eye(32, dtype=np.float32), (3, 1))
    fblob[0:32, 32:34] = rep(bn1_g, bn1_b)
    fblob[0:64, 34:36] = rep(bn2_g, bn2_b)
    fblob[0:64, 36:38] = rep(bn3_g, bn3_b)
    fblob[:, 38:42] = np.concatenate([
        np.stack([np.asarray(bnf1_g)[:128], np.asarray(bnf1_b)[:128]], 1),
        np.stack([np.asarray(bnf1_g)[128:], np.asarray(bnf1_b)[128:]], 1)],
        axis=1).astype(np.float32)
    fblob[0:32, 42:44] = np.stack(
        [np.asarray(bnf2_g), np.asarray(bnf2_b)], axis=1).astype(np.float32)
    fblob[0:32, 44:53] = wf3r
    fblob[0:1, 53:62] = b3r
    fblob[0:32, 62:94] = np.eye(32, dtype=np.float32)
    fblob[0:1, 94:126] = 1.0
    fblob[:, 126:158] = np.tile(np.eye(32, dtype=np.float32), (4, 1))
    fblob[:, 160:288] = np.eye(128, dtype=np.float32)
    fblob[0:1, 288:416] = 1.0
    return {
        "bfblob": bfblob.astype(bf16),
        "fblob": fblob,
        "wf1": wf1r.astype(bf16),
    }


def _prep_xr(xc):
    """-> xin [128, 8*31*128]: rows 32g..32g+32 = group g of 8 imgs:
    24 xr k-rows + 8 zero spacer rows, free laid out (img, y, x) so
    each group DMA is one contiguous-per-partition transfer."""
    out = np.empty((xc.shape[0], 24, 31, 128), dtype=bf16)
    for c in range(3):
        for dy in range(8):
            out[:, c * 8 + dy] = xc[:, c, dy:dy + 121:4, :].astype(bf16)
    xin = np.zeros((128, 8 * 31 * 128), dtype=bf16)
    for g in range(4):
        blk = out[8 * g:8 * g + 8].transpose(1, 0, 2, 3)   # [24,8,31,128]
        xin[32 * g:32 * g + 24] = blk.reshape(24, -1)
    return xin


_NC_CACHE = None


def kernel(**inputs):
    global _NC_CACHE
    if _NC_CACHE is None:
        _NC_CACHE = build_program()
    nc = _NC_CACHE

    x = np.asarray(inputs["x"])
    consts = _prep_consts(
        inputs["conv1_w"], inputs["conv2_w"], inputs["conv3_w"],
        inputs["fc1_w"], inputs["fc2_w"], inputs["fc3_w"], inputs["fc3_b"],
        inputs["bn1_g"], inputs["bn1_b"], inputs["bn2_g"], inputs["bn2_b"],
        inputs["bn3_g"], inputs["bn3_b"],
        inputs["bnf1_g"], inputs["bnf1_b"], inputs["bnf2_g"], inputs["bnf2_b"])

    in_maps = []
    for i in range(NCORES):
        m = dict(consts)
        m["xin"] = _prep_xr(x[Bc * i:Bc * (i + 1)])
        in_maps.append(m)

    res = run_bass_kernel_spmd(nc, in_maps, list(range(NCORES)))
    return np.asarray(res.results[0]["out"]).astype(np.float32)


if __name__ == "__main__":
    nc = build_program()
    print("program built ok")

